# revision 56
# baseline (speedup 1.0000x reference)
"""Trainium2 Bass kernel for nn_Attention_29935922053658 (sparse frame attention).

Sharding: data-parallel over batch B=8 -> 8 NeuronCores (1 batch each).

v2.6 design notes (v1 baseline 413us -> v2.2 317us -> v2.6 230us):
- Host supplies x TRANSPOSED and pre-cast to bf16 (xt [512, 3137]) plus
  bf16 weights with the q-scale folded in: no PE transposes, no on-chip
  x/weight casts, half the input DMA traffic.
- kq tiles hold per-frame columns [196 frame | cls | 59 zero-pad]; score
  matmuls run with full/trimmed stationaries so exp sees no garbage that
  matters (pad keys have zero v and zero ones-column entries).
- The cls token rides the frame attention as a 197th QUERY column: its
  scores/exp/av happen inside the per-head matmuls. Per-frame (num|den)
  columns are stashed by DMA and reduced at the end; the 16x over-counted
  cls self-term is removed with a host-computed -15*self correction.
- Denominators ride the v_ext ones-column (65th av output row); each
  head's po drains once into a per-frame staging strip S (bf16), from
  which DMA gathers denominators (one DMA/frame) and gpsimd (idle DSP)
  stages the unnormalized attnT.
- Normalization: one reciprocal per frame + ind8 rank-8 broadcast matmul
  + 4 in-place DVE muls on the seq-wide attnT buffer.
- Out-projection runs over GLOBAL 128-token chunks (25 instead of 33),
  bias folded in as a rank-1 5th contraction term.
- PE stream is software-pipelined: projection matmuls for pair P+1 are
  fillers between attention matmuls of pair P (keeps 2.4GHz p-state).
"""

import sys
import types
import json

for _p in ("/opt/trn_rl_repo", "/root/.axon_site"):
    if _p not in sys.path:
        sys.path.insert(0, _p)

import numpy as np

# ---------------------------------------------------------------------------
# Environment shims (required under the axon-proxied PJRT runtime):
#  1. antenv.axon_hooks registry (missing in this image) so trace=True can work.
#  2. Split >1 sync-waits off instructions — this walrus build's CoreV3
#     codegen rejects them ("Too many sync wait commands").
#  3. upload_artifacts: no artifact bucket in this container.
# ---------------------------------------------------------------------------


def _install_shims():
    import antenv

    if "antenv.axon_hooks" not in sys.modules:
        m = types.ModuleType("antenv.axon_hooks")
        m._hook = None

        def set_axon_ntff_profile_hook(h):
            m._hook = h

        def get_axon_ntff_profile_hook():
            return m._hook

        m.set_axon_ntff_profile_hook = set_axon_ntff_profile_hook
        m.get_axon_ntff_profile_hook = get_axon_ntff_profile_hook
        sys.modules["antenv.axon_hooks"] = m
        antenv.axon_hooks = m
        try:
            from trn_agent_boot.trn_boot import _ntff_profile_via_ctypes

            hook = _ntff_profile_via_ctypes("/opt/axon/libaxon_pjrt.so")
            if hook is not None:
                m._hook = hook
        except Exception:
            pass

    import concourse.bass_utils as bu
    import concourse.bass2jax as b2j

    if not getattr(bu, "_drain_patch_installed", False):
        bu._drain_patch_installed = True
        bu.upload_artifacts = lambda tmpdir: "local://" + str(tmpdir)

        _orig = b2j.compile_bir_kernel

        def _patched_compile(ant_bir_str, compile_dir, neff_name="file.neff"):
            d = json.loads(ant_bir_str)
            changed = False
            for fn in d.get("functions", []):
                for blk in fn.get("blocks", []):
                    insts = blk.get("instructions", [])
                    out = []
                    for ins in insts:
                        si = ins.get("sync_info") or {}
                        waits = si.get("on_wait") or []
                        if len(waits) > 1:
                            for ci, w in enumerate(waits[:-1]):
                                out.append(
                                    {
                                        "debug": ins.get("debug", 0),
                                        "engine": ins["engine"],
                                        "ins": [],
                                        "outs": [],
                                        "name": ins["name"] + f"-ws{ci}",
                                        "opcode": "NoOp",
                                        "sync_info": {
                                            "on_update": [],
                                            "on_wait": [w],
                                        },
                                    }
                                )
                            si["on_wait"] = waits[-1:]
                            changed = True
                        out.append(ins)
                    blk["instructions"] = out
            if changed:
                ant_bir_str = json.dumps(d).encode()
            return _orig(ant_bir_str, compile_dir, neff_name=neff_name)

        b2j.compile_bir_kernel = _patched_compile


_install_shims()

import concourse.bass as bass
import concourse.mybir as mybir
import concourse.tile as tile
from concourse.bass_utils import run_bass_kernel_spmd

f32 = mybir.dt.float32
bf16 = mybir.dt.bfloat16
AF = mybir.ActivationFunctionType

# Problem constants (hardcoded per spec)
N_SEQ = 3137
DIM = 512
H = 8
DH = 64
F = 16
NF = 196  # tokens per frame
NQ = 197  # queries per score block (frame tokens + cls)
NK = 197  # keys per frame block (frame + cls)
NKP = 256  # padded keys per frame block in kq tiles
N_CORES = 8
NTOK = N_SEQ - 1  # 3136 frame tokens
N_OCH = (NTOK + 127) // 128  # 25 global out-proj chunks (last = 64 tokens)


def build_kernel(with_bias=True):
    nc = bass.Bass()
    xt_d = nc.dram_tensor("xt", [DIM, N_SEQ], bf16, kind="ExternalInput")
    wqk_d = nc.dram_tensor("wqk", [DIM, 2 * DIM], bf16, kind="ExternalInput")
    wv_d = nc.dram_tensor("wv", [DIM, DIM], bf16, kind="ExternalInput")
    wout_d = nc.dram_tensor("wout", [DIM, DIM], bf16, kind="ExternalInput")
    bout_d = nc.dram_tensor("bout", [1, DIM], f32, kind="ExternalInput")
    ident_d = nc.dram_tensor("ident", [128, 128], bf16, kind="ExternalInput")
    ind8_d = nc.dram_tensor("ind8", [8, DIM], bf16, kind="ExternalInput")
    # -15 * (cls self-term): rows 0:64 = exp(s_self_h)*v_cls_h, row 64 =
    # exp(s_self_h); column h per head. Host-computed.
    neg15_d = nc.dram_tensor("neg15", [65, 8], f32, kind="ExternalInput")
    out_d = nc.dram_tensor("out", [N_SEQ, DIM], f32, kind="ExternalOutput")

    NBUF = 3  # manual rotation depth for kq
    NVBUF = 4  # v_ext rotation depth

    with tile.TileContext(nc) as tc:
        with (
            tc.tile_pool(name="const", bufs=1) as cpool,
            tc.tile_pool(name="scr", bufs=2) as scr,
            tc.tile_pool(name="ps_s", bufs=3, space="PSUM") as ps_s_pool,
            tc.tile_pool(name="ps_poo", bufs=3, space="PSUM") as ps_poo_pool,
            tc.tile_pool(name="ps_misc", bufs=2, space="PSUM") as ps_misc_pool,
        ):
            # ---------------- persistent SBUF tensors ----------------
            xT = []
            for c in range(4):
                t = cpool.tile([128, N_SEQ], bf16, name=f"xT{c}", tag=f"xT{c}")
                nc.sync.dma_start(out=t[:], in_=xt_d[c * 128 : (c + 1) * 128, :])
                xT.append(t)
            wqk = []
            for c in range(4):
                t = cpool.tile([128, 2 * DIM], bf16, name=f"wqk{c}", tag=f"wqk{c}")
                nc.sync.dma_start(out=t[:], in_=wqk_d[c * 128 : (c + 1) * 128, :])
                wqk.append(t)
            wv = []
            for c in range(4):
                t = cpool.tile([128, DIM], bf16, name=f"wv{c}", tag=f"wv{c}")
                nc.sync.dma_start(out=t[:], in_=wv_d[c * 128 : (c + 1) * 128, :])
                wv.append(t)
            wout = []
            for c in range(4):
                t = cpool.tile([128, DIM], bf16, name=f"wout{c}", tag=f"wout{c}")
                nc.sync.dma_start(out=t[:], in_=wout_d[c * 128 : (c + 1) * 128, :])
                wout.append(t)
            bout_sb = cpool.tile([1, DIM], f32, name="bout", tag="bout")
            nc.sync.dma_start(out=bout_sb[:], in_=bout_d[:])
            ident = cpool.tile([128, 128], bf16, name="ident", tag="ident")
            nc.sync.dma_start(out=ident[:], in_=ident_d[:])
            ind8 = cpool.tile([8, DIM], bf16, name="ind8", tag="ind8")
            nc.sync.dma_start(out=ind8[:], in_=ind8_d[:])
            neg15 = cpool.tile([65, 8], f32, name="neg15", tag="neg15")
            nc.sync.dma_start(out=neg15[:], in_=neg15_d[:])

            ones_row_bf = cpool.tile([1, 128], bf16, name="ones_row_bf", tag="orbf")
            nc.gpsimd.memset(ones_row_bf[:], 1.0)
            bout_bf = cpool.tile([1, DIM], bf16, name="bout_bf", tag="bout_bf")
            nc.vector.tensor_copy(bout_bf[:], bout_sb[:])

            # global attnT buffers: [128 inner dims (2 heads), NTOK] bf16
            attnT = [
                cpool.tile([128, NTOK], bf16, name=f"attnT{c}", tag=f"attnT{c}")
                for c in range(4)
            ]

            # rotating buffer sets (manual, so one-time inits survive reuse)
            kq_sets = [
                [
                    cpool.tile([128, 2, NKP], bf16, name=f"kq{s}_{m}", tag=f"kq{s}_{m}")
                    for m in range(8)
                ]
                for s in range(NBUF)
            ]
            v_sets = [
                (
                    cpool.tile([128, 8 * 65], bf16, name=f"v0_{s}", tag=f"v0_{s}"),
                    cpool.tile([128, 8 * 65], bf16, name=f"v1_{s}", tag=f"v1_{s}"),
                )
                for s in range(NVBUF)
            ]
            aT_sets = [
                cpool.tile([128, 2 * NQ], bf16, name=f"aT{s}", tag=f"aT{s}")
                for s in range(12)
            ]

            # S: per-frame staging strip [65, 8*197] bf16: per head
            # (attn-out rows 0:64 | den row 64) x (196 frame q | cls q col)
            S_sets = [
                cpool.tile([65, 8 * NQ], bf16, name=f"S_{p}", tag=f"S_{p}")
                for p in range(2)
            ]
            # cls stash: per-frame [65, 8] slices, reduced at epilogue
            stash = cpool.tile([65, F * 8], bf16, name="stash", tag="stash")
            s8_sets = [
                cpool.tile([8, NF], bf16, name=f"s8_{s}", tag=f"s8_{s}")
                for s in range(2)
            ]
            rs8_sets = [
                cpool.tile([8, NF], bf16, name=f"rs8_{s}", tag=f"rs8_{s}")
                for s in range(2)
            ]
            lden_sets = [
                cpool.tile([8, NF], f32, name=f"lden_{s}", tag=f"lden_{s}")
                for s in range(2)
            ]

            # ---------------- one-time inits on rotating sets ----------------
            for s in range(NVBUF):
                v0, v1 = v_sets[s]
                nc.gpsimd.memset(v1[64:128, :], 0.0)
                nc.gpsimd.memset(
                    v0[:].rearrange("p (h c) -> p h c", c=65)[:, :, 64:65], 1.0
                )
                nc.gpsimd.memset(
                    v1[0:69, :].rearrange("p (h c) -> p h c", c=65)[:, :, 64:65], 1.0
                )
            for s in range(NBUF):
                for m in range(4, 8):
                    for fl in range(2):
                        nc.gpsimd.memset(kq_sets[s][m][:, fl, NK:NKP], 0.0)

            # ---------------- preamble: cls q/k/v projections ----------------
            # v_cls row [1, 512] fp32
            v_cls = cpool.tile([1, DIM], f32, name="v_cls", tag="v_cls")
            ps = ps_misc_pool.tile([1, DIM], f32, name="ps_vc", tag="misc")
            for c in range(4):
                nc.tensor.matmul(
                    ps[:], lhsT=xT[c][:, 0:1], rhs=wv[c][:],
                    start=(c == 0), stop=(c == 3),
                )
            nc.vector.tensor_copy(v_cls[:], ps[:])

            # qkT_cls[m]: [128, 1] bf16 (transposed cls q/k per 128-dim chunk)
            qkT_cls = []
            for m in range(8):
                ps = ps_misc_pool.tile([128, 1], f32, name="ps_qt", tag="misc")
                for c in range(4):
                    nc.tensor.matmul(
                        ps[:],
                        lhsT=wqk[c][:, m * 128 : (m + 1) * 128],
                        rhs=xT[c][:, 0:1],
                        start=(c == 0),
                        stop=(c == 3),
                    )
                t = cpool.tile([128, 1], bf16, name=f"qkTc{m}", tag=f"qkTc{m}")
                nc.vector.tensor_copy(t[:], ps[:])
                qkT_cls.append(t)

            # one-time: cls q and k columns into every kq buffer set (col 196)
            for s in range(NBUF):
                for m in range(8):
                    for fl in range(2):
                        nc.gpsimd.tensor_copy(
                            kq_sets[s][m][:, fl, NF : NF + 1], qkT_cls[m][:]
                        )

            # v_ext_cls [1, 520] bf16: cls v + per-head ones; one-time row 68
            v_ext_cls = cpool.tile([1, 8 * 65], bf16, name="v_ext_cls", tag="vec")
            nc.gpsimd.memset(
                v_ext_cls[:].rearrange("p (h c) -> p h c", c=65)[:, :, 64:65], 1.0
            )
            nc.vector.tensor_copy(
                v_ext_cls[:].rearrange("p (h c) -> p h c", c=65)[:, :, 0:64],
                v_cls[:].rearrange("p (h c) -> p h c", c=64),
            )
            for s in range(NVBUF):
                nc.sync.dma_start(out=v_sets[s][1][68:69, :], in_=v_ext_cls[:])

            # ---------------- emit helpers ----------------
            def emit_kq_proj_group(fp, m):
                pr0 = 1 + fp * 2 * NF
                ps = ps_misc_pool.tile([128, 2 * NF], f32, name="ps_kq", tag="misc")
                for c in range(4):
                    nc.tensor.matmul(
                        ps[:],
                        lhsT=wqk[c][:, m * 128 : (m + 1) * 128],
                        rhs=xT[c][:, pr0 : pr0 + 2 * NF],
                        start=(c == 0),
                        stop=(c == 3),
                    )
                return ps

            def emit_kq_copy(ps, m, dst_set, use_act):
                kqt = kq_sets[dst_set][m]
                dst = kqt[:, :, 0:NF]
                src = ps[:].rearrange("p (a b) -> p a b", b=NF)
                if use_act:
                    nc.scalar.copy(dst, src)
                else:
                    nc.vector.tensor_copy(dst, src)

            def emit_v_proj(f):
                r0 = 1 + f * NF
                out_ps = []
                for t, (t0, tn) in enumerate(((0, 128), (128, 68))):
                    ps = ps_misc_pool.tile([tn, DIM], f32, name="ps_v", tag="misc")
                    for c in range(4):
                        nc.tensor.matmul(
                            ps[:],
                            lhsT=xT[c][:, r0 + t0 : r0 + t0 + tn],
                            rhs=wv[c][:],
                            start=(c == 0),
                            stop=(c == 3),
                        )
                    out_ps.append((ps, t, tn))
                return out_ps

            def emit_v_copy(ps, t, tn, vset):
                vx = v_sets[vset][t]
                nc.vector.tensor_copy(
                    vx[0:tn, :].rearrange("p (h c) -> p h c", c=65)[:, :, 0:64],
                    ps[:].rearrange("p (h c) -> p h c", c=64),
                )

            och_emitted = [False] * N_OCH

            def ready_ochunks(f_done):
                lim = (f_done + 1) * NF
                out = []
                for k in range(N_OCH):
                    if not och_emitted[k] and min((k + 1) * 128, NTOK) <= lim:
                        och_emitted[k] = True
                        out.append(k)
                return out

            def emit_outproj(k):
                t0 = k * 128
                tn = min(128, NTOK - t0)
                ps = ps_poo_pool.tile([tn, DIM], f32, name="ps_o", tag="poo")
                for c in range(4):
                    nc.tensor.matmul(
                        ps[:],
                        lhsT=attnT[c][:, t0 : t0 + tn],
                        rhs=wout[c][:],
                        start=(c == 0),
                        stop=(c == 3 and not with_bias),
                    )
                if with_bias:
                    nc.tensor.matmul(
                        ps[:], lhsT=ones_row_bf[0:1, 0:tn], rhs=bout_bf[:],
                        start=False, stop=True,
                    )
                o_sb = scr.tile([tn, DIM], f32, name="osb", tag=f"osb{k % 2}")
                if k % 2 == 0:
                    nc.scalar.copy(o_sb[:], ps[:])
                else:
                    nc.vector.tensor_copy(o_sb[:], ps[:])
                nc.sync.dma_start(out=out_d[1 + t0 : 1 + t0 + tn, :], in_=o_sb[:])

            def emit_recip(f, s8t, rs8t):
                # 1/x as exp(-ln(x)) on the Act engine: ~0.7us vs 1.37us on
                # DVE, and off the busy DVE queue
                ld = lden_sets[f % 2]
                nc.scalar.activation(ld[:], s8t[:], AF.Ln)
                nc.scalar.activation(rs8t[:], ld[:], AF.Exp, scale=-1.0)

            def emit_norm(f, s8t, rs8t):
                t0 = f * NF
                for c in range(4):
                    ps_r = ps_misc_pool.tile([128, NF], f32, name="ps_r", tag="misc")
                    nc.tensor.matmul(
                        ps_r[:],
                        lhsT=ind8[:, c * 128 : (c + 1) * 128],
                        rhs=rs8t[:],
                        start=True,
                        stop=True,
                    )
                    nc.vector.tensor_mul(
                        attnT[c][:, t0 : t0 + NF],
                        attnT[c][:, t0 : t0 + NF],
                        ps_r[:],
                    )

            # ---------------- software-pipelined main loop ----------------
            # prologue: project pair 0
            for m in range(8):
                ps = emit_kq_proj_group(0, m)
                emit_kq_copy(ps, m, 0, use_act=(m % 2 == 0))
            for f in (0, 1):
                for ps, t, tn in emit_v_proj(f):
                    emit_v_copy(ps, t, tn, f % NVBUF)

            pend_norm = None

            for f in range(F):
                fp, fl = f // 2, f % 2
                kset = fp % NBUF
                vset = f % NVBUF
                s8t = s8_sets[f % 2]
                rs8t = rs8_sets[f % 2]
                St = S_sets[f % 2]
                Sv = St[:].rearrange("p (h q) -> p h q", q=NQ)

                fillers = []
                if fp + 1 < F // 2:
                    nset = (fp + 1) % NBUF
                    ms = range(4) if fl == 0 else range(4, 8)
                    for m in ms:
                        fillers.append(("kq", m, nset))
                if f + 2 < F:
                    fillers.append(("v", f + 2, (f + 2) % NVBUF))

                def pop_filler():
                    if not fillers:
                        return
                    kind = fillers.pop(0)
                    if kind[0] == "kq":
                        _, m, nset = kind
                        ps = emit_kq_proj_group(fp + 1, m)
                        emit_kq_copy(ps, m, nset, use_act=(m % 2 == 0))
                    else:
                        _, vf, vs = kind
                        for ps, t, tn in emit_v_proj(vf):
                            emit_v_copy(ps, t, tn, vs)

                v0, v1 = v_sets[vset]
                aT_h = [None] * 8

                # start the previous frame's reciprocal ASAP so the norm
                # broadcast matmuls (emitted mid-frame) never stall the PE
                if pend_norm is not None:
                    emit_recip(*pend_norm)

                def emit_scores(h):
                    r = (h % 2) * 64
                    c = h // 2
                    ps = ps_s_pool.tile([128, 2 * NQ], f32, name="ps_sc", tag="sc")
                    nc.tensor.matmul(
                        ps[:, 0:NQ],
                        lhsT=kq_sets[kset][4 + c][r : r + 64, fl, 0:128],
                        rhs=kq_sets[kset][c][r : r + 64, fl, 0:NQ],
                        start=True,
                        stop=True,
                    )
                    # keys 128:198 = 68 frame + cls + 1 pad (trim stationary)
                    nc.tensor.matmul(
                        ps[0:70, NQ : 2 * NQ],
                        lhsT=kq_sets[kset][4 + c][r : r + 64, fl, 128:198],
                        rhs=kq_sets[kset][c][r : r + 64, fl, 0:NQ],
                        start=True,
                        stop=True,
                    )
                    aT = aT_sets[(f % 3) * 4 + (h % 4)]
                    nc.scalar.activation(aT[:], ps[:], AF.Exp)
                    aT_h[h] = aT

                def emit_av(h):
                    aT = aT_h[h]
                    po = ps_poo_pool.tile([65, NQ], f32, name="po", tag="poo")
                    nc.tensor.matmul(
                        po[:], lhsT=v0[:, h * 65 : (h + 1) * 65], rhs=aT[:, 0:NQ],
                        start=True, stop=False,
                    )
                    nc.tensor.matmul(
                        po[:], lhsT=v1[:, h * 65 : (h + 1) * 65],
                        rhs=aT[:, NQ : 2 * NQ],
                        start=False, stop=True,
                    )
                    # drain into the frame staging strip
                    if h % 2 == 0:
                        nc.scalar.copy(Sv[:, h, :], po[:])
                    else:
                        nc.vector.tensor_copy(Sv[:, h, :], po[:])
                    # stage unnormalized attnT on the idle gpsimd DSP
                    r = (h % 2) * 64
                    c = h // 2
                    nc.gpsimd.tensor_copy(
                        attnT[c][r : r + 64, f * NF : (f + 1) * NF],
                        Sv[0:64, h, 0:NF],
                    )

                pop_filler()
                emit_scores(0)
                emit_scores(1)
                emit_scores(2)
                pop_filler()
                emit_av(0)
                emit_scores(3)
                pop_filler()
                emit_av(1)
                emit_scores(4)
                if pend_norm is not None:
                    emit_norm(*pend_norm)
                    pend_norm = None
                emit_av(2)
                emit_scores(5)
                pop_filler()
                emit_av(3)
                emit_scores(6)
                pop_filler()
                emit_av(4)
                emit_scores(7)
                emit_av(5)
                emit_av(6)
                emit_av(7)
                # out-proj after the frame's po allocations so its psum-ring
                # slot never blocks an av matmul mid-frame
                if f >= 1:
                    for k in ready_ochunks(f - 1):
                        emit_outproj(k)
                while fillers:
                    pop_filler()

                # per-frame gathers: denominators [8, 196] and cls (num|den)
                # columns [65, 8] -> stash slice
                nc.sync.dma_start(out=s8t[:], in_=Sv[64:65, :, 0:NF])
                nc.sync.dma_start(
                    out=stash[:].rearrange("p (f h) -> p f h", h=8)[:, f, :],
                    in_=Sv[:, :, NF],
                )
                pend_norm = (f, s8t, rs8t)

            emit_recip(*pend_norm)
            emit_norm(*pend_norm)
            for k in ready_ochunks(F - 1):
                emit_outproj(k)

            # ---------------- cls epilogue ----------------
            acc65 = scr.tile([65, 8], f32, name="acc65", tag="acc65")
            nc.vector.tensor_copy(acc65[:], neg15[:])
            for f0 in range(F):
                nc.vector.tensor_add(
                    acc65[:], acc65[:],
                    stash[:].rearrange("p (a h) -> p a h", h=8)[:, f0, :],
                )
            accT = ps_misc_pool.tile([8, 65], bf16, name="accT", tag="misc")
            acc_bf = scr.tile([65, 8], bf16, name="acc_bf", tag="acc_bf")
            nc.vector.tensor_copy(acc_bf[:], acc65[:])
            nc.tensor.transpose(accT[:], acc_bf[:], ident[0:65, 0:65])
            accT_sb = scr.tile([8, 65], f32, name="accT_sb", tag="accT_sb")
            nc.vector.tensor_copy(accT_sb[:], accT[:])
            rden = scr.tile([8, 1], f32, name="rden", tag="rden")
            nc.vector.reciprocal(rden[:], accT_sb[:, 64:65])
            cls_n = scr.tile([8, 64], bf16, name="cls_n", tag="cls_n")
            nc.vector.tensor_scalar_mul(cls_n[:], accT_sb[:, 0:64], rden[:, 0:1])
            ps_t = ps_misc_pool.tile([64, 8], bf16, name="ps_t", tag="misc")
            nc.tensor.transpose(ps_t[:], cls_n[:], ident[0:8, 0:8])
            attnT_cls = [
                scr.tile([128, 1], bf16, name=f"aTc{c}", tag=f"aTc{c}")
                for c in range(4)
            ]
            for h in range(8):
                nc.vector.tensor_copy(
                    attnT_cls[h // 2][(h % 2) * 64 : (h % 2) * 64 + 64, :],
                    ps_t[:, h : h + 1],
                )
            ps_oc = ps_poo_pool.tile([1, DIM], f32, name="ps_oc", tag="poo")
            for c in range(4):
                nc.tensor.matmul(
                    ps_oc[:], lhsT=attnT_cls[c][:], rhs=wout[c][:],
                    start=(c == 0), stop=(c == 3),
                )
            o_cls = scr.tile([1, DIM], f32, name="o_cls", tag="o_cls")
            nc.vector.tensor_add(o_cls[:], ps_oc[:], bout_sb[:])
            nc.sync.dma_start(out=out_d[0:1, :], in_=o_cls[:])

    return nc


_NC_CACHE = {}


def _get_nc(with_bias):
    key = f"nc{int(with_bias)}"
    if key not in _NC_CACHE:
        _NC_CACHE[key] = build_kernel(with_bias)
    return _NC_CACHE[key]


def kernel(x, Wqkv, Wout, bout, f, _trace=False, _trace_kwargs=None):
    assert int(f) == F, f"kernel hardcoded for f={F}, got {f}"
    import ml_dtypes

    x = np.asarray(x, np.float32)
    Wqkv_s = np.asarray(Wqkv, np.float32).copy()
    Wqkv_s[:, :DIM] *= DH ** -0.5  # fold q scaling into the projection
    wqk_np = np.ascontiguousarray(Wqkv_s[:, : 2 * DIM]).astype(ml_dtypes.bfloat16)
    wv_np = np.ascontiguousarray(Wqkv_s[:, 2 * DIM :]).astype(ml_dtypes.bfloat16)
    wout_np = np.asarray(Wout, np.float32).astype(ml_dtypes.bfloat16)
    bout2 = np.asarray(bout, np.float32).reshape(1, DIM)

    ident_np = np.eye(128, dtype=ml_dtypes.bfloat16)
    ind8_np = np.zeros((8, DIM), dtype=ml_dtypes.bfloat16)
    for k in range(8):
        ind8_np[k, k * 64 : (k + 1) * 64] = 1.0

    xt_all = np.ascontiguousarray(x.transpose(0, 2, 1)).astype(ml_dtypes.bfloat16)

    # host-computed -15 * cls self-term per batch (fp32): the device counts
    # the cls self-attention term once per frame block (16x); subtract 15.
    x_cls = x[:, 0, :]  # [B, 512]
    q_cls = x_cls @ Wqkv_s[:, :DIM]  # scaled q  [B, 512]
    k_cls = x_cls @ Wqkv_s[:, DIM : 2 * DIM]
    v_cls = x_cls @ Wqkv_s[:, 2 * DIM :]
    qh = q_cls.reshape(-1, 8, 64)
    kh = k_cls.reshape(-1, 8, 64)
    vh = v_cls.reshape(-1, 8, 64)
    s_self = np.einsum("bhd,bhd->bh", qh, kh)  # [B, 8]
    e_self = np.exp(s_self)
    neg15_np = np.zeros((x.shape[0], 65, 8), np.float32)
    neg15_np[:, 0:64, :] = -15.0 * (e_self[:, None, :] * vh.transpose(0, 2, 1))
    neg15_np[:, 64, :] = -15.0 * e_self

    with_bias = bool(np.any(np.asarray(bout)))
    nc = _get_nc(with_bias)
    in_maps = [
        {
            "xt": xt_all[i],
            "wqk": wqk_np,
            "wv": wv_np,
            "wout": wout_np,
            "bout": bout2,
            "ident": ident_np,
            "ind8": ind8_np,
            "neg15": neg15_np[i],
        }
        for i in range(N_CORES)
    ]
    res = run_bass_kernel_spmd(
        nc,
        in_maps,
        list(range(N_CORES)),
        trace=_trace,
        **(_trace_kwargs or {}),
    )
    out = np.stack([res.results[i]["out"] for i in range(N_CORES)], axis=0)
    if _trace:
        kernel.last_results = res
    return out


# revision 58
# speedup vs baseline: 1.1801x; 1.1801x over previous
"""Trainium2 Bass kernel for nn_Attention_29935922053658 (sparse frame attention).

Sharding: data-parallel over batch B=8 -> 8 NeuronCores (1 batch each).

v2.6 design notes (v1 baseline 413us -> v2.2 317us -> v2.6 230us):
- Host supplies x TRANSPOSED and pre-cast to bf16 (xt [512, 3137]) plus
  bf16 weights with the q-scale folded in: no PE transposes, no on-chip
  x/weight casts, half the input DMA traffic.
- kq tiles hold per-frame columns [196 frame | cls | 59 zero-pad]; score
  matmuls run with full/trimmed stationaries so exp sees no garbage that
  matters (pad keys have zero v and zero ones-column entries).
- The cls token rides the frame attention as a 197th QUERY column: its
  scores/exp/av happen inside the per-head matmuls. Per-frame (num|den)
  columns are stashed by DMA and reduced at the end; the 16x over-counted
  cls self-term is removed with a host-computed -15*self correction.
- Denominators ride the v_ext ones-column (65th av output row); each
  head's po drains once into a per-frame staging strip S (bf16), from
  which DMA gathers denominators (one DMA/frame) and gpsimd (idle DSP)
  stages the unnormalized attnT.
- Normalization: one reciprocal per frame + ind8 rank-8 broadcast matmul
  + 4 in-place DVE muls on the seq-wide attnT buffer.
- Out-projection runs over GLOBAL 128-token chunks (25 instead of 33),
  bias folded in as a rank-1 5th contraction term.
- PE stream is software-pipelined: projection matmuls for pair P+1 are
  fillers between attention matmuls of pair P (keeps 2.4GHz p-state).
"""

import sys
import types
import json

for _p in ("/opt/trn_rl_repo", "/root/.axon_site"):
    if _p not in sys.path:
        sys.path.insert(0, _p)

import numpy as np

# ---------------------------------------------------------------------------
# Environment shims (required under the axon-proxied PJRT runtime):
#  1. antenv.axon_hooks registry (missing in this image) so trace=True can work.
#  2. Split >1 sync-waits off instructions — this walrus build's CoreV3
#     codegen rejects them ("Too many sync wait commands").
#  3. upload_artifacts: no artifact bucket in this container.
# ---------------------------------------------------------------------------


def _install_shims():
    import antenv

    if "antenv.axon_hooks" not in sys.modules:
        m = types.ModuleType("antenv.axon_hooks")
        m._hook = None

        def set_axon_ntff_profile_hook(h):
            m._hook = h

        def get_axon_ntff_profile_hook():
            return m._hook

        m.set_axon_ntff_profile_hook = set_axon_ntff_profile_hook
        m.get_axon_ntff_profile_hook = get_axon_ntff_profile_hook
        sys.modules["antenv.axon_hooks"] = m
        antenv.axon_hooks = m
        try:
            from trn_agent_boot.trn_boot import _ntff_profile_via_ctypes

            hook = _ntff_profile_via_ctypes("/opt/axon/libaxon_pjrt.so")
            if hook is not None:
                m._hook = hook
        except Exception:
            pass

    import concourse.bass_utils as bu
    import concourse.bass2jax as b2j

    if not getattr(bu, "_drain_patch_installed", False):
        bu._drain_patch_installed = True
        bu.upload_artifacts = lambda tmpdir: "local://" + str(tmpdir)

        _orig = b2j.compile_bir_kernel

        def _patched_compile(ant_bir_str, compile_dir, neff_name="file.neff"):
            d = json.loads(ant_bir_str)
            changed = False
            for fn in d.get("functions", []):
                for blk in fn.get("blocks", []):
                    insts = blk.get("instructions", [])
                    out = []
                    for ins in insts:
                        si = ins.get("sync_info") or {}
                        waits = si.get("on_wait") or []
                        if len(waits) > 1:
                            for ci, w in enumerate(waits[:-1]):
                                out.append(
                                    {
                                        "debug": ins.get("debug", 0),
                                        "engine": ins["engine"],
                                        "ins": [],
                                        "outs": [],
                                        "name": ins["name"] + f"-ws{ci}",
                                        "opcode": "NoOp",
                                        "sync_info": {
                                            "on_update": [],
                                            "on_wait": [w],
                                        },
                                    }
                                )
                            si["on_wait"] = waits[-1:]
                            changed = True
                        out.append(ins)
                    blk["instructions"] = out
            if changed:
                ant_bir_str = json.dumps(d).encode()
            return _orig(ant_bir_str, compile_dir, neff_name=neff_name)

        b2j.compile_bir_kernel = _patched_compile


_install_shims()

import concourse.bass as bass
import concourse.mybir as mybir
import concourse.tile as tile
from concourse.bass_utils import run_bass_kernel_spmd

f32 = mybir.dt.float32
bf16 = mybir.dt.bfloat16
AF = mybir.ActivationFunctionType

# Problem constants (hardcoded per spec)
N_SEQ = 3137
DIM = 512
H = 8
DH = 64
F = 16
NF = 196  # tokens per frame
NQ = 197  # queries per score block (frame tokens + cls)
NK = 197  # keys per frame block (frame + cls)
NKP = 256  # padded keys per frame block in kq tiles
N_CORES = 8
NTOK = N_SEQ - 1  # 3136 frame tokens
N_OCH = (NTOK + 127) // 128  # 25 global out-proj chunks (last = 64 tokens)


def build_kernel(with_bias=True):
    nc = bass.Bass()
    xt_d = nc.dram_tensor("xt", [DIM, N_SEQ], bf16, kind="ExternalInput")
    wqk_d = nc.dram_tensor("wqk", [DIM, 2 * DIM], bf16, kind="ExternalInput")
    wv_d = nc.dram_tensor("wv", [DIM, DIM], bf16, kind="ExternalInput")
    wout_d = nc.dram_tensor("wout", [DIM, DIM], bf16, kind="ExternalInput")
    bout_d = nc.dram_tensor("bout", [1, DIM], f32, kind="ExternalInput")
    ident_d = nc.dram_tensor("ident", [128, 128], bf16, kind="ExternalInput")
    ind8_d = nc.dram_tensor("ind8", [8, DIM], bf16, kind="ExternalInput")
    # -15 * (cls self-term): rows 0:64 = exp(s_self_h)*v_cls_h, row 64 =
    # exp(s_self_h); column h per head. Host-computed.
    neg15_d = nc.dram_tensor("neg15", [65, 8], f32, kind="ExternalInput")
    out_d = nc.dram_tensor("out", [N_SEQ, DIM], f32, kind="ExternalOutput")

    NBUF = 3  # manual rotation depth for kq
    NVBUF = 4  # v_ext rotation depth

    with tile.TileContext(nc) as tc:
        with (
            tc.tile_pool(name="const", bufs=1) as cpool,
            tc.tile_pool(name="scr", bufs=2) as scr,
            tc.tile_pool(name="ps_s", bufs=3, space="PSUM") as ps_s_pool,
            tc.tile_pool(name="ps_poo", bufs=3, space="PSUM") as ps_poo_pool,
            tc.tile_pool(name="ps_misc", bufs=2, space="PSUM") as ps_misc_pool,
        ):
            # ---------------- persistent SBUF tensors ----------------
            xT = []
            for c in range(4):
                t = cpool.tile([128, N_SEQ], bf16, name=f"xT{c}", tag=f"xT{c}")
                nc.sync.dma_start(out=t[:], in_=xt_d[c * 128 : (c + 1) * 128, :])
                xT.append(t)
            wqk = []
            for c in range(4):
                t = cpool.tile([128, 2 * DIM], bf16, name=f"wqk{c}", tag=f"wqk{c}")
                nc.sync.dma_start(out=t[:], in_=wqk_d[c * 128 : (c + 1) * 128, :])
                wqk.append(t)
            wv = []
            for c in range(4):
                t = cpool.tile([128, DIM], bf16, name=f"wv{c}", tag=f"wv{c}")
                nc.sync.dma_start(out=t[:], in_=wv_d[c * 128 : (c + 1) * 128, :])
                wv.append(t)
            wout = []
            for c in range(4):
                t = cpool.tile([128, DIM], bf16, name=f"wout{c}", tag=f"wout{c}")
                nc.sync.dma_start(out=t[:], in_=wout_d[c * 128 : (c + 1) * 128, :])
                wout.append(t)
            bout_sb = cpool.tile([1, DIM], f32, name="bout", tag="bout")
            nc.sync.dma_start(out=bout_sb[:], in_=bout_d[:])
            ident = cpool.tile([128, 128], bf16, name="ident", tag="ident")
            nc.sync.dma_start(out=ident[:], in_=ident_d[:])
            ind8 = cpool.tile([8, DIM], bf16, name="ind8", tag="ind8")
            nc.sync.dma_start(out=ind8[:], in_=ind8_d[:])
            neg15 = cpool.tile([65, 8], f32, name="neg15", tag="neg15")
            nc.sync.dma_start(out=neg15[:], in_=neg15_d[:])

            ones_row_bf = cpool.tile([1, 128], bf16, name="ones_row_bf", tag="orbf")
            nc.gpsimd.memset(ones_row_bf[:], 1.0)
            bout_bf = cpool.tile([1, DIM], bf16, name="bout_bf", tag="bout_bf")
            nc.vector.tensor_copy(bout_bf[:], bout_sb[:])

            # global attnT buffers: [128 inner dims (2 heads), NTOK] bf16
            attnT = [
                cpool.tile([128, NTOK], bf16, name=f"attnT{c}", tag=f"attnT{c}")
                for c in range(4)
            ]

            # rotating buffer sets (manual, so one-time inits survive reuse)
            kq_sets = [
                [
                    cpool.tile([128, 2, NKP], bf16, name=f"kq{s}_{m}", tag=f"kq{s}_{m}")
                    for m in range(8)
                ]
                for s in range(NBUF)
            ]
            v_sets = [
                (
                    cpool.tile([128, 8 * 65], bf16, name=f"v0_{s}", tag=f"v0_{s}"),
                    cpool.tile([128, 8 * 65], bf16, name=f"v1_{s}", tag=f"v1_{s}"),
                )
                for s in range(NVBUF)
            ]
            aT_sets = [
                cpool.tile([128, 2 * NQ], bf16, name=f"aT{s}", tag=f"aT{s}")
                for s in range(12)
            ]

            # S: per-frame staging strip [65, 8*197] bf16: per head
            # (attn-out rows 0:64 | den row 64) x (196 frame q | cls q col)
            S_sets = [
                cpool.tile([65, 8 * NQ], bf16, name=f"S_{p}", tag=f"S_{p}")
                for p in range(2)
            ]
            # cls stash: per-frame [65, 8] slices, reduced at epilogue
            stash = cpool.tile([65, F * 8], bf16, name="stash", tag="stash")
            s8_sets = [
                cpool.tile([8, NF], bf16, name=f"s8_{s}", tag=f"s8_{s}")
                for s in range(2)
            ]
            rs8_sets = [
                cpool.tile([8, NF], bf16, name=f"rs8_{s}", tag=f"rs8_{s}")
                for s in range(2)
            ]
            lden_sets = [
                cpool.tile([8, NF], f32, name=f"lden_{s}", tag=f"lden_{s}")
                for s in range(2)
            ]

            # ---------------- one-time inits on rotating sets ----------------
            for s in range(NVBUF):
                v0, v1 = v_sets[s]
                nc.gpsimd.memset(v1[64:128, :], 0.0)
                nc.gpsimd.memset(
                    v0[:].rearrange("p (h c) -> p h c", c=65)[:, :, 64:65], 1.0
                )
                nc.gpsimd.memset(
                    v1[0:69, :].rearrange("p (h c) -> p h c", c=65)[:, :, 64:65], 1.0
                )
            for s in range(NBUF):
                for m in range(4, 8):
                    for fl in range(2):
                        nc.gpsimd.memset(kq_sets[s][m][:, fl, NK:NKP], 0.0)

            # ---------------- emit helpers ----------------
            def emit_kq_proj_group(fp, m):
                pr0 = 1 + fp * 2 * NF
                ps = ps_misc_pool.tile([128, 2 * NF], f32, name="ps_kq", tag="misc")
                for c in range(4):
                    nc.tensor.matmul(
                        ps[:],
                        lhsT=wqk[c][:, m * 128 : (m + 1) * 128],
                        rhs=xT[c][:, pr0 : pr0 + 2 * NF],
                        start=(c == 0),
                        stop=(c == 3),
                    )
                return ps

            def emit_kq_copy(ps, m, dst_set, use_act):
                kqt = kq_sets[dst_set][m]
                dst = kqt[:, :, 0:NF]
                src = ps[:].rearrange("p (a b) -> p a b", b=NF)
                if use_act:
                    nc.scalar.copy(dst, src)
                else:
                    nc.vector.tensor_copy(dst, src)

            def emit_v_proj(f):
                r0 = 1 + f * NF
                out_ps = []
                for t, (t0, tn) in enumerate(((0, 128), (128, 68))):
                    ps = ps_misc_pool.tile([tn, DIM], f32, name="ps_v", tag="misc")
                    for c in range(4):
                        nc.tensor.matmul(
                            ps[:],
                            lhsT=xT[c][:, r0 + t0 : r0 + t0 + tn],
                            rhs=wv[c][:],
                            start=(c == 0),
                            stop=(c == 3),
                        )
                    out_ps.append((ps, t, tn))
                return out_ps

            def emit_v_copy(ps, t, tn, vset):
                vx = v_sets[vset][t]
                nc.vector.tensor_copy(
                    vx[0:tn, :].rearrange("p (h c) -> p h c", c=65)[:, :, 0:64],
                    ps[:].rearrange("p (h c) -> p h c", c=64),
                )

            och_emitted = [False] * N_OCH

            def ready_ochunks(f_done):
                lim = (f_done + 1) * NF
                out = []
                for k in range(N_OCH):
                    if not och_emitted[k] and min((k + 1) * 128, NTOK) <= lim:
                        och_emitted[k] = True
                        out.append(k)
                return out

            def emit_outproj(k):
                t0 = k * 128
                tn = min(128, NTOK - t0)
                ps = ps_poo_pool.tile([tn, DIM], f32, name="ps_o", tag="poo")
                for c in range(4):
                    nc.tensor.matmul(
                        ps[:],
                        lhsT=attnT[c][:, t0 : t0 + tn],
                        rhs=wout[c][:],
                        start=(c == 0),
                        stop=(c == 3 and not with_bias),
                    )
                if with_bias:
                    nc.tensor.matmul(
                        ps[:], lhsT=ones_row_bf[0:1, 0:tn], rhs=bout_bf[:],
                        start=False, stop=True,
                    )
                o_sb = scr.tile([tn, DIM], f32, name="osb", tag=f"osb{k % 2}")
                if k % 2 == 0:
                    nc.scalar.copy(o_sb[:], ps[:])
                else:
                    nc.vector.tensor_copy(o_sb[:], ps[:])
                nc.sync.dma_start(out=out_d[1 + t0 : 1 + t0 + tn, :], in_=o_sb[:])

            def emit_recip(f, s8t, rs8t):
                # 1/x as exp(-ln(x)) on the Act engine: ~0.7us vs 1.37us on
                # DVE, and off the busy DVE queue
                ld = lden_sets[f % 2]
                nc.scalar.activation(ld[:], s8t[:], AF.Ln)
                nc.scalar.activation(rs8t[:], ld[:], AF.Exp, scale=-1.0)

            def emit_norm(f, s8t, rs8t):
                t0 = f * NF
                for c in range(4):
                    ps_r = ps_misc_pool.tile([128, NF], f32, name="ps_r", tag="misc")
                    nc.tensor.matmul(
                        ps_r[:],
                        lhsT=ind8[:, c * 128 : (c + 1) * 128],
                        rhs=rs8t[:],
                        start=True,
                        stop=True,
                    )
                    nc.vector.tensor_mul(
                        attnT[c][:, t0 : t0 + NF],
                        attnT[c][:, t0 : t0 + NF],
                        ps_r[:],
                    )

            # prologue: project pair 0 first (big 392-row groups ramp the PE)
            for m in range(8):
                ps = emit_kq_proj_group(0, m)
                emit_kq_copy(ps, m, 0, use_act=(m % 2 == 0))
            for f in (0, 1):
                for ps, t, tn in emit_v_proj(f):
                    emit_v_copy(ps, t, tn, f % NVBUF)

            # ---------------- software-pipelined main loop ----------------

            # ---------------- preamble: cls q/k/v projections ----------------
            # v_cls row [1, 512] fp32
            v_cls = cpool.tile([1, DIM], f32, name="v_cls", tag="v_cls")
            ps = ps_misc_pool.tile([1, DIM], f32, name="ps_vc", tag="misc")
            for c in range(4):
                nc.tensor.matmul(
                    ps[:], lhsT=xT[c][:, 0:1], rhs=wv[c][:],
                    start=(c == 0), stop=(c == 3),
                )
            nc.vector.tensor_copy(v_cls[:], ps[:])

            # qkT_cls[m]: [128, 1] bf16 (transposed cls q/k per 128-dim chunk)
            qkT_cls = []
            for m in range(8):
                ps = ps_misc_pool.tile([128, 1], f32, name="ps_qt", tag="misc")
                for c in range(4):
                    nc.tensor.matmul(
                        ps[:],
                        lhsT=wqk[c][:, m * 128 : (m + 1) * 128],
                        rhs=xT[c][:, 0:1],
                        start=(c == 0),
                        stop=(c == 3),
                    )
                t = cpool.tile([128, 1], bf16, name=f"qkTc{m}", tag=f"qkTc{m}")
                nc.vector.tensor_copy(t[:], ps[:])
                qkT_cls.append(t)

            # one-time: cls q and k columns into every kq buffer set (col 196)
            for s in range(NBUF):
                for m in range(8):
                    for fl in range(2):
                        nc.gpsimd.tensor_copy(
                            kq_sets[s][m][:, fl, NF : NF + 1], qkT_cls[m][:]
                        )

            # v_ext_cls [1, 520] bf16: cls v + per-head ones; one-time row 68
            v_ext_cls = cpool.tile([1, 8 * 65], bf16, name="v_ext_cls", tag="vec")
            nc.gpsimd.memset(
                v_ext_cls[:].rearrange("p (h c) -> p h c", c=65)[:, :, 64:65], 1.0
            )
            nc.vector.tensor_copy(
                v_ext_cls[:].rearrange("p (h c) -> p h c", c=65)[:, :, 0:64],
                v_cls[:].rearrange("p (h c) -> p h c", c=64),
            )
            for s in range(NVBUF):
                nc.sync.dma_start(out=v_sets[s][1][68:69, :], in_=v_ext_cls[:])


            pend_norm = None

            for f in range(F):
                fp, fl = f // 2, f % 2
                kset = fp % NBUF
                vset = f % NVBUF
                s8t = s8_sets[f % 2]
                rs8t = rs8_sets[f % 2]
                St = S_sets[f % 2]
                Sv = St[:].rearrange("p (h q) -> p h q", q=NQ)

                fillers = []
                if fp + 1 < F // 2:
                    nset = (fp + 1) % NBUF
                    ms = range(4) if fl == 0 else range(4, 8)
                    for m in ms:
                        fillers.append(("kq", m, nset))
                if f + 2 < F:
                    fillers.append(("v", f + 2, (f + 2) % NVBUF))

                def pop_filler():
                    if not fillers:
                        return
                    kind = fillers.pop(0)
                    if kind[0] == "kq":
                        _, m, nset = kind
                        ps = emit_kq_proj_group(fp + 1, m)
                        emit_kq_copy(ps, m, nset, use_act=(m % 2 == 0))
                    else:
                        _, vf, vs = kind
                        for ps, t, tn in emit_v_proj(vf):
                            emit_v_copy(ps, t, tn, vs)

                v0, v1 = v_sets[vset]
                aT_h = [None] * 8

                # start the previous frame's reciprocal ASAP so the norm
                # broadcast matmuls (emitted mid-frame) never stall the PE
                if pend_norm is not None:
                    emit_recip(*pend_norm)

                def emit_scores(h):
                    r = (h % 2) * 64
                    c = h // 2
                    ps = ps_s_pool.tile([128, 2 * NQ], f32, name="ps_sc", tag="sc")
                    nc.tensor.matmul(
                        ps[:, 0:NQ],
                        lhsT=kq_sets[kset][4 + c][r : r + 64, fl, 0:128],
                        rhs=kq_sets[kset][c][r : r + 64, fl, 0:NQ],
                        start=True,
                        stop=True,
                    )
                    # keys 128:198 = 68 frame + cls + 1 pad (trim stationary)
                    nc.tensor.matmul(
                        ps[0:70, NQ : 2 * NQ],
                        lhsT=kq_sets[kset][4 + c][r : r + 64, fl, 128:198],
                        rhs=kq_sets[kset][c][r : r + 64, fl, 0:NQ],
                        start=True,
                        stop=True,
                    )
                    aT = aT_sets[(f % 3) * 4 + (h % 4)]
                    nc.scalar.activation(aT[:], ps[:], AF.Exp)
                    aT_h[h] = aT

                def emit_av(h):
                    aT = aT_h[h]
                    po = ps_poo_pool.tile([65, NQ], f32, name="po", tag="poo")
                    nc.tensor.matmul(
                        po[:], lhsT=v0[:, h * 65 : (h + 1) * 65], rhs=aT[:, 0:NQ],
                        start=True, stop=False,
                    )
                    nc.tensor.matmul(
                        po[:], lhsT=v1[:, h * 65 : (h + 1) * 65],
                        rhs=aT[:, NQ : 2 * NQ],
                        start=False, stop=True,
                    )
                    # drain into the frame staging strip
                    if h % 2 == 0:
                        nc.scalar.copy(Sv[:, h, :], po[:])
                    else:
                        nc.vector.tensor_copy(Sv[:, h, :], po[:])
                    # stage unnormalized attnT on the idle gpsimd DSP
                    r = (h % 2) * 64
                    c = h // 2
                    nc.gpsimd.tensor_copy(
                        attnT[c][r : r + 64, f * NF : (f + 1) * NF],
                        Sv[0:64, h, 0:NF],
                    )

                pop_filler()
                emit_scores(0)
                emit_scores(1)
                emit_scores(2)
                pop_filler()
                emit_av(0)
                emit_scores(3)
                pop_filler()
                emit_av(1)
                emit_scores(4)
                if pend_norm is not None:
                    emit_norm(*pend_norm)
                    pend_norm = None
                emit_av(2)
                emit_scores(5)
                pop_filler()
                emit_av(3)
                emit_scores(6)
                pop_filler()
                emit_av(4)
                emit_scores(7)
                emit_av(5)
                emit_av(6)
                emit_av(7)
                # out-proj after the frame's po allocations so its psum-ring
                # slot never blocks an av matmul mid-frame
                if f >= 1:
                    for k in ready_ochunks(f - 1):
                        emit_outproj(k)
                while fillers:
                    pop_filler()

                # per-frame gathers: denominators [8, 196] and cls (num|den)
                # columns [65, 8] -> stash slice
                nc.sync.dma_start(out=s8t[:], in_=Sv[64:65, :, 0:NF])
                nc.sync.dma_start(
                    out=stash[:].rearrange("p (f h) -> p f h", h=8)[:, f, :],
                    in_=Sv[:, :, NF],
                )
                pend_norm = (f, s8t, rs8t)

            emit_recip(*pend_norm)
            emit_norm(*pend_norm)
            for k in ready_ochunks(F - 1):
                emit_outproj(k)

            # ---------------- cls epilogue ----------------
            acc65 = scr.tile([65, 8], f32, name="acc65", tag="acc65")
            nc.vector.tensor_copy(acc65[:], neg15[:])
            for f0 in range(F):
                nc.vector.tensor_add(
                    acc65[:], acc65[:],
                    stash[:].rearrange("p (a h) -> p a h", h=8)[:, f0, :],
                )
            accT = ps_misc_pool.tile([8, 65], bf16, name="accT", tag="misc")
            acc_bf = scr.tile([65, 8], bf16, name="acc_bf", tag="acc_bf")
            nc.vector.tensor_copy(acc_bf[:], acc65[:])
            nc.tensor.transpose(accT[:], acc_bf[:], ident[0:65, 0:65])
            accT_sb = scr.tile([8, 65], f32, name="accT_sb", tag="accT_sb")
            nc.vector.tensor_copy(accT_sb[:], accT[:])
            rden = scr.tile([8, 1], f32, name="rden", tag="rden")
            nc.vector.reciprocal(rden[:], accT_sb[:, 64:65])
            cls_n = scr.tile([8, 64], bf16, name="cls_n", tag="cls_n")
            nc.vector.tensor_scalar_mul(cls_n[:], accT_sb[:, 0:64], rden[:, 0:1])
            ps_t = ps_misc_pool.tile([64, 8], bf16, name="ps_t", tag="misc")
            nc.tensor.transpose(ps_t[:], cls_n[:], ident[0:8, 0:8])
            attnT_cls = [
                scr.tile([128, 1], bf16, name=f"aTc{c}", tag=f"aTc{c}")
                for c in range(4)
            ]
            for h in range(8):
                nc.vector.tensor_copy(
                    attnT_cls[h // 2][(h % 2) * 64 : (h % 2) * 64 + 64, :],
                    ps_t[:, h : h + 1],
                )
            ps_oc = ps_poo_pool.tile([1, DIM], f32, name="ps_oc", tag="poo")
            for c in range(4):
                nc.tensor.matmul(
                    ps_oc[:], lhsT=attnT_cls[c][:], rhs=wout[c][:],
                    start=(c == 0), stop=(c == 3),
                )
            o_cls = scr.tile([1, DIM], f32, name="o_cls", tag="o_cls")
            nc.vector.tensor_add(o_cls[:], ps_oc[:], bout_sb[:])
            nc.sync.dma_start(out=out_d[0:1, :], in_=o_cls[:])

    return nc


_NC_CACHE = {}


def _get_nc(with_bias):
    key = f"nc{int(with_bias)}"
    if key not in _NC_CACHE:
        _NC_CACHE[key] = build_kernel(with_bias)
    return _NC_CACHE[key]


def kernel(x, Wqkv, Wout, bout, f, _trace=False, _trace_kwargs=None):
    assert int(f) == F, f"kernel hardcoded for f={F}, got {f}"
    import ml_dtypes

    x = np.asarray(x, np.float32)
    Wqkv_s = np.asarray(Wqkv, np.float32).copy()
    Wqkv_s[:, :DIM] *= DH ** -0.5  # fold q scaling into the projection
    wqk_np = np.ascontiguousarray(Wqkv_s[:, : 2 * DIM]).astype(ml_dtypes.bfloat16)
    wv_np = np.ascontiguousarray(Wqkv_s[:, 2 * DIM :]).astype(ml_dtypes.bfloat16)
    wout_np = np.asarray(Wout, np.float32).astype(ml_dtypes.bfloat16)
    bout2 = np.asarray(bout, np.float32).reshape(1, DIM)

    ident_np = np.eye(128, dtype=ml_dtypes.bfloat16)
    ind8_np = np.zeros((8, DIM), dtype=ml_dtypes.bfloat16)
    for k in range(8):
        ind8_np[k, k * 64 : (k + 1) * 64] = 1.0

    xt_all = np.ascontiguousarray(x.transpose(0, 2, 1)).astype(ml_dtypes.bfloat16)

    # host-computed -15 * cls self-term per batch (fp32): the device counts
    # the cls self-attention term once per frame block (16x); subtract 15.
    x_cls = x[:, 0, :]  # [B, 512]
    q_cls = x_cls @ Wqkv_s[:, :DIM]  # scaled q  [B, 512]
    k_cls = x_cls @ Wqkv_s[:, DIM : 2 * DIM]
    v_cls = x_cls @ Wqkv_s[:, 2 * DIM :]
    qh = q_cls.reshape(-1, 8, 64)
    kh = k_cls.reshape(-1, 8, 64)
    vh = v_cls.reshape(-1, 8, 64)
    s_self = np.einsum("bhd,bhd->bh", qh, kh)  # [B, 8]
    e_self = np.exp(s_self)
    neg15_np = np.zeros((x.shape[0], 65, 8), np.float32)
    neg15_np[:, 0:64, :] = -15.0 * (e_self[:, None, :] * vh.transpose(0, 2, 1))
    neg15_np[:, 64, :] = -15.0 * e_self

    with_bias = bool(np.any(np.asarray(bout)))
    nc = _get_nc(with_bias)
    in_maps = [
        {
            "xt": xt_all[i],
            "wqk": wqk_np,
            "wv": wv_np,
            "wout": wout_np,
            "bout": bout2,
            "ident": ident_np,
            "ind8": ind8_np,
            "neg15": neg15_np[i],
        }
        for i in range(N_CORES)
    ]
    res = run_bass_kernel_spmd(
        nc,
        in_maps,
        list(range(N_CORES)),
        trace=_trace,
        **(_trace_kwargs or {}),
    )
    out = np.stack([res.results[i]["out"] for i in range(N_CORES)], axis=0)
    if _trace:
        kernel.last_results = res
    return out


# revision 59
# speedup vs baseline: 1.1952x; 1.0128x over previous
"""Trainium2 Bass kernel for nn_Attention_29935922053658 (sparse frame attention).

Sharding: data-parallel over batch B=8 -> 8 NeuronCores (1 batch each).

v2.6 design notes (v1 baseline 413us -> v2.2 317us -> v2.6 230us):
- Host supplies x TRANSPOSED and pre-cast to bf16 (xt [512, 3137]) plus
  bf16 weights with the q-scale folded in: no PE transposes, no on-chip
  x/weight casts, half the input DMA traffic.
- kq tiles hold per-frame columns [196 frame | cls | 59 zero-pad]; score
  matmuls run with full/trimmed stationaries so exp sees no garbage that
  matters (pad keys have zero v and zero ones-column entries).
- The cls token rides the frame attention as a 197th QUERY column: its
  scores/exp/av happen inside the per-head matmuls. Per-frame (num|den)
  columns are stashed by DMA and reduced at the end; the 16x over-counted
  cls self-term is removed with a host-computed -15*self correction.
- Denominators ride the v_ext ones-column (65th av output row); each
  head's po drains once into a per-frame staging strip S (bf16), from
  which DMA gathers denominators (one DMA/frame) and gpsimd (idle DSP)
  stages the unnormalized attnT.
- Normalization: one reciprocal per frame + ind8 rank-8 broadcast matmul
  + 4 in-place DVE muls on the seq-wide attnT buffer.
- Out-projection runs over GLOBAL 128-token chunks (25 instead of 33),
  bias folded in as a rank-1 5th contraction term.
- PE stream is software-pipelined: projection matmuls for pair P+1 are
  fillers between attention matmuls of pair P (keeps 2.4GHz p-state).
"""

import sys
import types
import json

for _p in ("/opt/trn_rl_repo", "/root/.axon_site"):
    if _p not in sys.path:
        sys.path.insert(0, _p)

import numpy as np

# ---------------------------------------------------------------------------
# Environment shims (required under the axon-proxied PJRT runtime):
#  1. antenv.axon_hooks registry (missing in this image) so trace=True can work.
#  2. Split >1 sync-waits off instructions — this walrus build's CoreV3
#     codegen rejects them ("Too many sync wait commands").
#  3. upload_artifacts: no artifact bucket in this container.
# ---------------------------------------------------------------------------


def _install_shims():
    import antenv

    if "antenv.axon_hooks" not in sys.modules:
        m = types.ModuleType("antenv.axon_hooks")
        m._hook = None

        def set_axon_ntff_profile_hook(h):
            m._hook = h

        def get_axon_ntff_profile_hook():
            return m._hook

        m.set_axon_ntff_profile_hook = set_axon_ntff_profile_hook
        m.get_axon_ntff_profile_hook = get_axon_ntff_profile_hook
        sys.modules["antenv.axon_hooks"] = m
        antenv.axon_hooks = m
        try:
            from trn_agent_boot.trn_boot import _ntff_profile_via_ctypes

            hook = _ntff_profile_via_ctypes("/opt/axon/libaxon_pjrt.so")
            if hook is not None:
                m._hook = hook
        except Exception:
            pass

    import concourse.bass_utils as bu
    import concourse.bass2jax as b2j

    if not getattr(bu, "_drain_patch_installed", False):
        bu._drain_patch_installed = True
        bu.upload_artifacts = lambda tmpdir: "local://" + str(tmpdir)

        _orig = b2j.compile_bir_kernel

        def _patched_compile(ant_bir_str, compile_dir, neff_name="file.neff"):
            d = json.loads(ant_bir_str)
            changed = False
            for fn in d.get("functions", []):
                for blk in fn.get("blocks", []):
                    insts = blk.get("instructions", [])
                    out = []
                    for ins in insts:
                        si = ins.get("sync_info") or {}
                        waits = si.get("on_wait") or []
                        if len(waits) > 1:
                            for ci, w in enumerate(waits[:-1]):
                                out.append(
                                    {
                                        "debug": ins.get("debug", 0),
                                        "engine": ins["engine"],
                                        "ins": [],
                                        "outs": [],
                                        "name": ins["name"] + f"-ws{ci}",
                                        "opcode": "NoOp",
                                        "sync_info": {
                                            "on_update": [],
                                            "on_wait": [w],
                                        },
                                    }
                                )
                            si["on_wait"] = waits[-1:]
                            changed = True
                        out.append(ins)
                    blk["instructions"] = out
            if changed:
                ant_bir_str = json.dumps(d).encode()
            return _orig(ant_bir_str, compile_dir, neff_name=neff_name)

        b2j.compile_bir_kernel = _patched_compile


_install_shims()

import concourse.bass as bass
import concourse.mybir as mybir
import concourse.tile as tile
from concourse.bass_utils import run_bass_kernel_spmd

f32 = mybir.dt.float32
bf16 = mybir.dt.bfloat16
AF = mybir.ActivationFunctionType

# Problem constants (hardcoded per spec)
N_SEQ = 3137
DIM = 512
H = 8
DH = 64
F = 16
NF = 196  # tokens per frame
NQ = 197  # queries per score block (frame tokens + cls)
NK = 197  # keys per frame block (frame + cls)
NKP = 256  # padded keys per frame block in kq tiles
N_CORES = 8
NTOK = N_SEQ - 1  # 3136 frame tokens
N_OCH = (NTOK + 127) // 128  # 25 global out-proj chunks (last = 64 tokens)


def build_kernel(with_bias=True):
    nc = bass.Bass()
    xt_d = nc.dram_tensor("xt", [DIM, N_SEQ], bf16, kind="ExternalInput")
    wqk_d = nc.dram_tensor("wqk", [DIM, 2 * DIM], bf16, kind="ExternalInput")
    wv_d = nc.dram_tensor("wv", [DIM, DIM], bf16, kind="ExternalInput")
    wout_d = nc.dram_tensor("wout", [DIM, DIM], bf16, kind="ExternalInput")
    bout_d = nc.dram_tensor("bout", [1, DIM], f32, kind="ExternalInput")
    ident_d = nc.dram_tensor("ident", [128, 128], bf16, kind="ExternalInput")
    ind8_d = nc.dram_tensor("ind8", [8, DIM], bf16, kind="ExternalInput")
    # -15 * (cls self-term): rows 0:64 = exp(s_self_h)*v_cls_h, row 64 =
    # exp(s_self_h); column h per head. Host-computed.
    neg15_d = nc.dram_tensor("neg15", [65, 8], f32, kind="ExternalInput")
    out_d = nc.dram_tensor("out", [N_SEQ, DIM], f32, kind="ExternalOutput")

    NBUF = 3  # manual rotation depth for kq
    NVBUF = 4  # v_ext rotation depth

    with tile.TileContext(nc) as tc:
        with (
            tc.tile_pool(name="const", bufs=1) as cpool,
            tc.tile_pool(name="scr", bufs=2) as scr,
            tc.tile_pool(name="ps_s", bufs=3, space="PSUM") as ps_s_pool,
            tc.tile_pool(name="ps_poo", bufs=3, space="PSUM") as ps_poo_pool,
            tc.tile_pool(name="ps_misc", bufs=2, space="PSUM") as ps_misc_pool,
        ):
            # ---------------- persistent SBUF tensors ----------------
            xT = []
            for c in range(4):
                t = cpool.tile([128, N_SEQ], bf16, name=f"xT{c}", tag=f"xT{c}")
                nc.sync.dma_start(out=t[:], in_=xt_d[c * 128 : (c + 1) * 128, :])
                xT.append(t)
            wqk = []
            for c in range(4):
                t = cpool.tile([128, 2 * DIM], bf16, name=f"wqk{c}", tag=f"wqk{c}")
                nc.sync.dma_start(out=t[:], in_=wqk_d[c * 128 : (c + 1) * 128, :])
                wqk.append(t)
            wv = []
            for c in range(4):
                t = cpool.tile([128, DIM], bf16, name=f"wv{c}", tag=f"wv{c}")
                nc.sync.dma_start(out=t[:], in_=wv_d[c * 128 : (c + 1) * 128, :])
                wv.append(t)
            wout = []
            for c in range(4):
                t = cpool.tile([128, DIM], bf16, name=f"wout{c}", tag=f"wout{c}")
                nc.sync.dma_start(out=t[:], in_=wout_d[c * 128 : (c + 1) * 128, :])
                wout.append(t)
            bout_sb = cpool.tile([1, DIM], f32, name="bout", tag="bout")
            nc.sync.dma_start(out=bout_sb[:], in_=bout_d[:])
            ident = cpool.tile([128, 128], bf16, name="ident", tag="ident")
            nc.sync.dma_start(out=ident[:], in_=ident_d[:])
            ind8 = cpool.tile([8, DIM], bf16, name="ind8", tag="ind8")
            nc.sync.dma_start(out=ind8[:], in_=ind8_d[:])
            neg15 = cpool.tile([65, 8], f32, name="neg15", tag="neg15")
            nc.sync.dma_start(out=neg15[:], in_=neg15_d[:])

            ones_row_bf = cpool.tile([1, 128], bf16, name="ones_row_bf", tag="orbf")
            nc.gpsimd.memset(ones_row_bf[:], 1.0)
            bout_bf = cpool.tile([1, DIM], bf16, name="bout_bf", tag="bout_bf")
            nc.vector.tensor_copy(bout_bf[:], bout_sb[:])

            # global attnT buffers: [128 inner dims (2 heads), NTOK] bf16
            attnT = [
                cpool.tile([128, NTOK], bf16, name=f"attnT{c}", tag=f"attnT{c}")
                for c in range(4)
            ]

            # rotating buffer sets (manual, so one-time inits survive reuse)
            kq_sets = [
                [
                    cpool.tile([128, 2, NKP], bf16, name=f"kq{s}_{m}", tag=f"kq{s}_{m}")
                    for m in range(8)
                ]
                for s in range(NBUF)
            ]
            v_sets = [
                (
                    cpool.tile([128, 8 * 65], bf16, name=f"v0_{s}", tag=f"v0_{s}"),
                    cpool.tile([128, 8 * 65], bf16, name=f"v1_{s}", tag=f"v1_{s}"),
                )
                for s in range(NVBUF)
            ]
            aT_sets = [
                cpool.tile([128, 2 * NQ], bf16, name=f"aT{s}", tag=f"aT{s}")
                for s in range(12)
            ]

            # S: per-frame staging strip [65, 8*197] bf16: per head
            # (attn-out rows 0:64 | den row 64) x (196 frame q | cls q col)
            S_sets = [
                cpool.tile([65, 8 * NQ], bf16, name=f"S_{p}", tag=f"S_{p}")
                for p in range(2)
            ]
            # cls stash: per-frame [65, 8] slices, reduced at epilogue
            stash = cpool.tile([65, F * 8], bf16, name="stash", tag="stash")
            s8_sets = [
                cpool.tile([8, NF], bf16, name=f"s8_{s}", tag=f"s8_{s}")
                for s in range(2)
            ]
            rs8_sets = [
                cpool.tile([8, NF], bf16, name=f"rs8_{s}", tag=f"rs8_{s}")
                for s in range(2)
            ]
            lden_sets = [
                cpool.tile([8, NF], f32, name=f"lden_{s}", tag=f"lden_{s}")
                for s in range(2)
            ]

            # ---------------- one-time inits on rotating sets ----------------
            for s in range(NVBUF):
                v0, v1 = v_sets[s]
                nc.gpsimd.memset(v1[64:128, :], 0.0)
                nc.gpsimd.memset(
                    v0[:].rearrange("p (h c) -> p h c", c=65)[:, :, 64:65], 1.0
                )
                nc.gpsimd.memset(
                    v1[0:69, :].rearrange("p (h c) -> p h c", c=65)[:, :, 64:65], 1.0
                )
            for s in range(NBUF):
                for m in range(4, 8):
                    for fl in range(2):
                        nc.gpsimd.memset(kq_sets[s][m][:, fl, NK:NKP], 0.0)

            # ---------------- emit helpers ----------------
            def emit_kq_proj_group(fp, m):
                pr0 = 1 + fp * 2 * NF
                ps = ps_misc_pool.tile([128, 2 * NF], f32, name="ps_kq", tag="misc")
                for c in range(4):
                    nc.tensor.matmul(
                        ps[:],
                        lhsT=wqk[c][:, m * 128 : (m + 1) * 128],
                        rhs=xT[c][:, pr0 : pr0 + 2 * NF],
                        start=(c == 0),
                        stop=(c == 3),
                    )
                return ps

            def emit_kq_copy(ps, m, dst_set, use_act):
                kqt = kq_sets[dst_set][m]
                dst = kqt[:, :, 0:NF]
                src = ps[:].rearrange("p (a b) -> p a b", b=NF)
                if use_act:
                    nc.scalar.copy(dst, src)
                else:
                    nc.vector.tensor_copy(dst, src)

            def emit_v_proj(f):
                r0 = 1 + f * NF
                out_ps = []
                for t, (t0, tn) in enumerate(((0, 128), (128, 68))):
                    ps = ps_misc_pool.tile([tn, DIM], f32, name="ps_v", tag="misc")
                    for c in range(4):
                        nc.tensor.matmul(
                            ps[:],
                            lhsT=xT[c][:, r0 + t0 : r0 + t0 + tn],
                            rhs=wv[c][:],
                            start=(c == 0),
                            stop=(c == 3),
                        )
                    out_ps.append((ps, t, tn))
                return out_ps

            def emit_v_copy(ps, t, tn, vset):
                vx = v_sets[vset][t]
                nc.vector.tensor_copy(
                    vx[0:tn, :].rearrange("p (h c) -> p h c", c=65)[:, :, 0:64],
                    ps[:].rearrange("p (h c) -> p h c", c=64),
                )

            och_emitted = [False] * N_OCH

            def ready_ochunks(f_done):
                lim = (f_done + 1) * NF
                out = []
                for k in range(N_OCH):
                    if not och_emitted[k] and min((k + 1) * 128, NTOK) <= lim:
                        och_emitted[k] = True
                        out.append(k)
                return out

            def emit_outproj(k):
                t0 = k * 128
                tn = min(128, NTOK - t0)
                ps = ps_poo_pool.tile([tn, DIM], f32, name="ps_o", tag="poo")
                for c in range(4):
                    nc.tensor.matmul(
                        ps[:],
                        lhsT=attnT[c][:, t0 : t0 + tn],
                        rhs=wout[c][:],
                        start=(c == 0),
                        stop=(c == 3 and not with_bias),
                    )
                if with_bias:
                    nc.tensor.matmul(
                        ps[:], lhsT=ones_row_bf[0:1, 0:tn], rhs=bout_bf[:],
                        start=False, stop=True,
                    )
                o_sb = scr.tile([tn, DIM], f32, name="osb", tag=f"osb{k % 2}")
                nc.vector.tensor_copy(o_sb[:], ps[:])
                nc.sync.dma_start(out=out_d[1 + t0 : 1 + t0 + tn, :], in_=o_sb[:])

            def emit_recip(f, s8t, rs8t):
                # 1/x as exp(-ln(x)) on the Act engine: ~0.7us vs 1.37us on
                # DVE, and off the busy DVE queue
                ld = lden_sets[f % 2]
                nc.scalar.activation(ld[:], s8t[:], AF.Ln)
                nc.scalar.activation(rs8t[:], ld[:], AF.Exp, scale=-1.0)

            def emit_norm(f, s8t, rs8t):
                t0 = f * NF
                for c in range(4):
                    ps_r = ps_misc_pool.tile([128, NF], f32, name="ps_r", tag="misc")
                    nc.tensor.matmul(
                        ps_r[:],
                        lhsT=ind8[:, c * 128 : (c + 1) * 128],
                        rhs=rs8t[:],
                        start=True,
                        stop=True,
                    )
                    nc.vector.tensor_mul(
                        attnT[c][:, t0 : t0 + NF],
                        attnT[c][:, t0 : t0 + NF],
                        ps_r[:],
                    )

            # prologue: project pair 0 first (big 392-row groups ramp the PE)
            for m in range(8):
                ps = emit_kq_proj_group(0, m)
                emit_kq_copy(ps, m, 0, use_act=False)
            for f in (0, 1):
                for ps, t, tn in emit_v_proj(f):
                    emit_v_copy(ps, t, tn, f % NVBUF)

            # ---------------- software-pipelined main loop ----------------

            # ---------------- preamble: cls q/k/v projections ----------------
            # v_cls row [1, 512] fp32
            v_cls = cpool.tile([1, DIM], f32, name="v_cls", tag="v_cls")
            ps = ps_misc_pool.tile([1, DIM], f32, name="ps_vc", tag="misc")
            for c in range(4):
                nc.tensor.matmul(
                    ps[:], lhsT=xT[c][:, 0:1], rhs=wv[c][:],
                    start=(c == 0), stop=(c == 3),
                )
            nc.vector.tensor_copy(v_cls[:], ps[:])

            # qkT_cls[m]: [128, 1] bf16 (transposed cls q/k per 128-dim chunk)
            qkT_cls = []
            for m in range(8):
                ps = ps_misc_pool.tile([128, 1], f32, name="ps_qt", tag="misc")
                for c in range(4):
                    nc.tensor.matmul(
                        ps[:],
                        lhsT=wqk[c][:, m * 128 : (m + 1) * 128],
                        rhs=xT[c][:, 0:1],
                        start=(c == 0),
                        stop=(c == 3),
                    )
                t = cpool.tile([128, 1], bf16, name=f"qkTc{m}", tag=f"qkTc{m}")
                nc.vector.tensor_copy(t[:], ps[:])
                qkT_cls.append(t)

            # one-time: cls q and k columns into every kq buffer set (col 196)
            for s in range(NBUF):
                for m in range(8):
                    for fl in range(2):
                        nc.gpsimd.tensor_copy(
                            kq_sets[s][m][:, fl, NF : NF + 1], qkT_cls[m][:]
                        )

            # v_ext_cls [1, 520] bf16: cls v + per-head ones; one-time row 68
            v_ext_cls = cpool.tile([1, 8 * 65], bf16, name="v_ext_cls", tag="vec")
            nc.gpsimd.memset(
                v_ext_cls[:].rearrange("p (h c) -> p h c", c=65)[:, :, 64:65], 1.0
            )
            nc.vector.tensor_copy(
                v_ext_cls[:].rearrange("p (h c) -> p h c", c=65)[:, :, 0:64],
                v_cls[:].rearrange("p (h c) -> p h c", c=64),
            )
            for s in range(NVBUF):
                nc.sync.dma_start(out=v_sets[s][1][68:69, :], in_=v_ext_cls[:])


            pend_norm = None

            for f in range(F):
                fp, fl = f // 2, f % 2
                kset = fp % NBUF
                vset = f % NVBUF
                s8t = s8_sets[f % 2]
                rs8t = rs8_sets[f % 2]
                St = S_sets[f % 2]
                Sv = St[:].rearrange("p (h q) -> p h q", q=NQ)

                fillers = []
                if fp + 1 < F // 2:
                    nset = (fp + 1) % NBUF
                    ms = range(4) if fl == 0 else range(4, 8)
                    for m in ms:
                        fillers.append(("kq", m, nset))
                if f + 2 < F:
                    fillers.append(("v", f + 2, (f + 2) % NVBUF))

                def pop_filler():
                    if not fillers:
                        return
                    kind = fillers.pop(0)
                    if kind[0] == "kq":
                        _, m, nset = kind
                        ps = emit_kq_proj_group(fp + 1, m)
                        emit_kq_copy(ps, m, nset, use_act=False)
                    else:
                        _, vf, vs = kind
                        for ps, t, tn in emit_v_proj(vf):
                            emit_v_copy(ps, t, tn, vs)

                v0, v1 = v_sets[vset]
                aT_h = [None] * 8

                # start the previous frame's reciprocal ASAP so the norm
                # broadcast matmuls (emitted mid-frame) never stall the PE
                if pend_norm is not None:
                    emit_recip(*pend_norm)

                def emit_scores(h):
                    r = (h % 2) * 64
                    c = h // 2
                    ps = ps_s_pool.tile([128, 2 * NQ], f32, name="ps_sc", tag="sc")
                    nc.tensor.matmul(
                        ps[:, 0:NQ],
                        lhsT=kq_sets[kset][4 + c][r : r + 64, fl, 0:128],
                        rhs=kq_sets[kset][c][r : r + 64, fl, 0:NQ],
                        start=True,
                        stop=True,
                    )
                    # keys 128:198 = 68 frame + cls + 1 pad (trim stationary)
                    nc.tensor.matmul(
                        ps[0:70, NQ : 2 * NQ],
                        lhsT=kq_sets[kset][4 + c][r : r + 64, fl, 128:198],
                        rhs=kq_sets[kset][c][r : r + 64, fl, 0:NQ],
                        start=True,
                        stop=True,
                    )
                    aT = aT_sets[(f % 3) * 4 + (h % 4)]
                    nc.scalar.activation(aT[:], ps[:], AF.Exp)
                    aT_h[h] = aT

                def emit_av(h):
                    aT = aT_h[h]
                    po = ps_poo_pool.tile([65, NQ], f32, name="po", tag="poo")
                    nc.tensor.matmul(
                        po[:], lhsT=v0[:, h * 65 : (h + 1) * 65], rhs=aT[:, 0:NQ],
                        start=True, stop=False,
                    )
                    nc.tensor.matmul(
                        po[:], lhsT=v1[:, h * 65 : (h + 1) * 65],
                        rhs=aT[:, NQ : 2 * NQ],
                        start=False, stop=True,
                    )
                    # drain into the frame staging strip
                    if h % 2 == 0:
                        nc.scalar.copy(Sv[:, h, :], po[:])
                    else:
                        nc.vector.tensor_copy(Sv[:, h, :], po[:])
                    # stage unnormalized attnT on the idle gpsimd DSP
                    r = (h % 2) * 64
                    c = h // 2
                    nc.gpsimd.tensor_copy(
                        attnT[c][r : r + 64, f * NF : (f + 1) * NF],
                        Sv[0:64, h, 0:NF],
                    )

                pop_filler()
                emit_scores(0)
                emit_scores(1)
                emit_scores(2)
                pop_filler()
                emit_av(0)
                emit_scores(3)
                pop_filler()
                emit_av(1)
                emit_scores(4)
                if pend_norm is not None:
                    emit_norm(*pend_norm)
                    pend_norm = None
                emit_av(2)
                emit_scores(5)
                pop_filler()
                emit_av(3)
                emit_scores(6)
                pop_filler()
                emit_av(4)
                emit_scores(7)
                emit_av(5)
                emit_av(6)
                emit_av(7)
                # out-proj after the frame's po allocations so its psum-ring
                # slot never blocks an av matmul mid-frame
                if f >= 1:
                    for k in ready_ochunks(f - 1):
                        emit_outproj(k)
                while fillers:
                    pop_filler()

                # per-frame gathers: denominators [8, 196] and cls (num|den)
                # columns [65, 8] -> stash slice
                nc.sync.dma_start(out=s8t[:], in_=Sv[64:65, :, 0:NF])
                nc.sync.dma_start(
                    out=stash[:].rearrange("p (f h) -> p f h", h=8)[:, f, :],
                    in_=Sv[:, :, NF],
                )
                pend_norm = (f, s8t, rs8t)

            emit_recip(*pend_norm)
            emit_norm(*pend_norm)
            for k in ready_ochunks(F - 1):
                emit_outproj(k)

            # ---------------- cls epilogue ----------------
            acc65 = scr.tile([65, 8], f32, name="acc65", tag="acc65")
            nc.vector.tensor_copy(acc65[:], neg15[:])
            for f0 in range(F):
                nc.vector.tensor_add(
                    acc65[:], acc65[:],
                    stash[:].rearrange("p (a h) -> p a h", h=8)[:, f0, :],
                )
            accT = ps_misc_pool.tile([8, 65], bf16, name="accT", tag="misc")
            acc_bf = scr.tile([65, 8], bf16, name="acc_bf", tag="acc_bf")
            nc.vector.tensor_copy(acc_bf[:], acc65[:])
            nc.tensor.transpose(accT[:], acc_bf[:], ident[0:65, 0:65])
            accT_sb = scr.tile([8, 65], f32, name="accT_sb", tag="accT_sb")
            nc.vector.tensor_copy(accT_sb[:], accT[:])
            rden = scr.tile([8, 1], f32, name="rden", tag="rden")
            nc.vector.reciprocal(rden[:], accT_sb[:, 64:65])
            cls_n = scr.tile([8, 64], bf16, name="cls_n", tag="cls_n")
            nc.vector.tensor_scalar_mul(cls_n[:], accT_sb[:, 0:64], rden[:, 0:1])
            ps_t = ps_misc_pool.tile([64, 8], bf16, name="ps_t", tag="misc")
            nc.tensor.transpose(ps_t[:], cls_n[:], ident[0:8, 0:8])
            attnT_cls = [
                scr.tile([128, 1], bf16, name=f"aTc{c}", tag=f"aTc{c}")
                for c in range(4)
            ]
            for h in range(8):
                nc.vector.tensor_copy(
                    attnT_cls[h // 2][(h % 2) * 64 : (h % 2) * 64 + 64, :],
                    ps_t[:, h : h + 1],
                )
            ps_oc = ps_poo_pool.tile([1, DIM], f32, name="ps_oc", tag="poo")
            for c in range(4):
                nc.tensor.matmul(
                    ps_oc[:], lhsT=attnT_cls[c][:], rhs=wout[c][:],
                    start=(c == 0), stop=(c == 3),
                )
            o_cls = scr.tile([1, DIM], f32, name="o_cls", tag="o_cls")
            nc.vector.tensor_add(o_cls[:], ps_oc[:], bout_sb[:])
            nc.sync.dma_start(out=out_d[0:1, :], in_=o_cls[:])

    return nc


_NC_CACHE = {}


def _get_nc(with_bias):
    key = f"nc{int(with_bias)}"
    if key not in _NC_CACHE:
        _NC_CACHE[key] = build_kernel(with_bias)
    return _NC_CACHE[key]


def kernel(x, Wqkv, Wout, bout, f, _trace=False, _trace_kwargs=None):
    assert int(f) == F, f"kernel hardcoded for f={F}, got {f}"
    import ml_dtypes

    x = np.asarray(x, np.float32)
    Wqkv_s = np.asarray(Wqkv, np.float32).copy()
    Wqkv_s[:, :DIM] *= DH ** -0.5  # fold q scaling into the projection
    wqk_np = np.ascontiguousarray(Wqkv_s[:, : 2 * DIM]).astype(ml_dtypes.bfloat16)
    wv_np = np.ascontiguousarray(Wqkv_s[:, 2 * DIM :]).astype(ml_dtypes.bfloat16)
    wout_np = np.asarray(Wout, np.float32).astype(ml_dtypes.bfloat16)
    bout2 = np.asarray(bout, np.float32).reshape(1, DIM)

    ident_np = np.eye(128, dtype=ml_dtypes.bfloat16)
    ind8_np = np.zeros((8, DIM), dtype=ml_dtypes.bfloat16)
    for k in range(8):
        ind8_np[k, k * 64 : (k + 1) * 64] = 1.0

    xt_all = np.ascontiguousarray(x.transpose(0, 2, 1)).astype(ml_dtypes.bfloat16)

    # host-computed -15 * cls self-term per batch (fp32): the device counts
    # the cls self-attention term once per frame block (16x); subtract 15.
    x_cls = x[:, 0, :]  # [B, 512]
    q_cls = x_cls @ Wqkv_s[:, :DIM]  # scaled q  [B, 512]
    k_cls = x_cls @ Wqkv_s[:, DIM : 2 * DIM]
    v_cls = x_cls @ Wqkv_s[:, 2 * DIM :]
    qh = q_cls.reshape(-1, 8, 64)
    kh = k_cls.reshape(-1, 8, 64)
    vh = v_cls.reshape(-1, 8, 64)
    s_self = np.einsum("bhd,bhd->bh", qh, kh)  # [B, 8]
    e_self = np.exp(s_self)
    neg15_np = np.zeros((x.shape[0], 65, 8), np.float32)
    neg15_np[:, 0:64, :] = -15.0 * (e_self[:, None, :] * vh.transpose(0, 2, 1))
    neg15_np[:, 64, :] = -15.0 * e_self

    with_bias = bool(np.any(np.asarray(bout)))
    nc = _get_nc(with_bias)
    in_maps = [
        {
            "xt": xt_all[i],
            "wqk": wqk_np,
            "wv": wv_np,
            "wout": wout_np,
            "bout": bout2,
            "ident": ident_np,
            "ind8": ind8_np,
            "neg15": neg15_np[i],
        }
        for i in range(N_CORES)
    ]
    res = run_bass_kernel_spmd(
        nc,
        in_maps,
        list(range(N_CORES)),
        trace=_trace,
        **(_trace_kwargs or {}),
    )
    out = np.stack([res.results[i]["out"] for i in range(N_CORES)], axis=0)
    if _trace:
        kernel.last_results = res
    return out


# revision 60
# speedup vs baseline: 1.2160x; 1.0174x over previous
"""Trainium2 Bass kernel for nn_Attention_29935922053658 (sparse frame attention).

Sharding: data-parallel over batch B=8 -> 8 NeuronCores (1 batch each).

v2.6 design notes (v1 baseline 413us -> v2.2 317us -> v2.6 230us):
- Host supplies x TRANSPOSED and pre-cast to bf16 (xt [512, 3137]) plus
  bf16 weights with the q-scale folded in: no PE transposes, no on-chip
  x/weight casts, half the input DMA traffic.
- kq tiles hold per-frame columns [196 frame | cls | 59 zero-pad]; score
  matmuls run with full/trimmed stationaries so exp sees no garbage that
  matters (pad keys have zero v and zero ones-column entries).
- The cls token rides the frame attention as a 197th QUERY column: its
  scores/exp/av happen inside the per-head matmuls. Per-frame (num|den)
  columns are stashed by DMA and reduced at the end; the 16x over-counted
  cls self-term is removed with a host-computed -15*self correction.
- Denominators ride the v_ext ones-column (65th av output row); each
  head's po drains once into a per-frame staging strip S (bf16), from
  which DMA gathers denominators (one DMA/frame) and gpsimd (idle DSP)
  stages the unnormalized attnT.
- Normalization: one reciprocal per frame + ind8 rank-8 broadcast matmul
  + 4 in-place DVE muls on the seq-wide attnT buffer.
- Out-projection runs over GLOBAL 128-token chunks (25 instead of 33),
  bias folded in as a rank-1 5th contraction term.
- PE stream is software-pipelined: projection matmuls for pair P+1 are
  fillers between attention matmuls of pair P (keeps 2.4GHz p-state).
"""

import sys
import types
import json

for _p in ("/opt/trn_rl_repo", "/root/.axon_site"):
    if _p not in sys.path:
        sys.path.insert(0, _p)

import numpy as np

# ---------------------------------------------------------------------------
# Environment shims (required under the axon-proxied PJRT runtime):
#  1. antenv.axon_hooks registry (missing in this image) so trace=True can work.
#  2. Split >1 sync-waits off instructions — this walrus build's CoreV3
#     codegen rejects them ("Too many sync wait commands").
#  3. upload_artifacts: no artifact bucket in this container.
# ---------------------------------------------------------------------------


def _install_shims():
    import antenv

    if "antenv.axon_hooks" not in sys.modules:
        m = types.ModuleType("antenv.axon_hooks")
        m._hook = None

        def set_axon_ntff_profile_hook(h):
            m._hook = h

        def get_axon_ntff_profile_hook():
            return m._hook

        m.set_axon_ntff_profile_hook = set_axon_ntff_profile_hook
        m.get_axon_ntff_profile_hook = get_axon_ntff_profile_hook
        sys.modules["antenv.axon_hooks"] = m
        antenv.axon_hooks = m
        try:
            from trn_agent_boot.trn_boot import _ntff_profile_via_ctypes

            hook = _ntff_profile_via_ctypes("/opt/axon/libaxon_pjrt.so")
            if hook is not None:
                m._hook = hook
        except Exception:
            pass

    import concourse.bass_utils as bu
    import concourse.bass2jax as b2j

    if not getattr(bu, "_drain_patch_installed", False):
        bu._drain_patch_installed = True
        bu.upload_artifacts = lambda tmpdir: "local://" + str(tmpdir)

        _orig = b2j.compile_bir_kernel

        def _patched_compile(ant_bir_str, compile_dir, neff_name="file.neff"):
            d = json.loads(ant_bir_str)
            changed = False
            for fn in d.get("functions", []):
                for blk in fn.get("blocks", []):
                    insts = blk.get("instructions", [])
                    out = []
                    for ins in insts:
                        si = ins.get("sync_info") or {}
                        waits = si.get("on_wait") or []
                        if len(waits) > 1:
                            for ci, w in enumerate(waits[:-1]):
                                out.append(
                                    {
                                        "debug": ins.get("debug", 0),
                                        "engine": ins["engine"],
                                        "ins": [],
                                        "outs": [],
                                        "name": ins["name"] + f"-ws{ci}",
                                        "opcode": "NoOp",
                                        "sync_info": {
                                            "on_update": [],
                                            "on_wait": [w],
                                        },
                                    }
                                )
                            si["on_wait"] = waits[-1:]
                            changed = True
                        out.append(ins)
                    blk["instructions"] = out
            if changed:
                ant_bir_str = json.dumps(d).encode()
            return _orig(ant_bir_str, compile_dir, neff_name=neff_name)

        b2j.compile_bir_kernel = _patched_compile


_install_shims()

import concourse.bass as bass
import concourse.mybir as mybir
import concourse.tile as tile
from concourse.bass_utils import run_bass_kernel_spmd

f32 = mybir.dt.float32
bf16 = mybir.dt.bfloat16
AF = mybir.ActivationFunctionType

# Problem constants (hardcoded per spec)
N_SEQ = 3137
DIM = 512
H = 8
DH = 64
F = 16
NF = 196  # tokens per frame
NQ = 197  # queries per score block (frame tokens + cls)
NK = 197  # keys per frame block (frame + cls)
NKP = 256  # padded keys per frame block in kq tiles
N_CORES = 8
NTOK = N_SEQ - 1  # 3136 frame tokens
N_OCH = (NTOK + 127) // 128  # 25 global out-proj chunks (last = 64 tokens)


def build_kernel(with_bias=True):
    nc = bass.Bass()
    xt_d = nc.dram_tensor("xt", [DIM, N_SEQ], bf16, kind="ExternalInput")
    wqk_d = nc.dram_tensor("wqk", [DIM, 2 * DIM], bf16, kind="ExternalInput")
    wv_d = nc.dram_tensor("wv", [DIM, DIM], bf16, kind="ExternalInput")
    wout_d = nc.dram_tensor("wout", [DIM, DIM], bf16, kind="ExternalInput")
    bout_d = nc.dram_tensor("bout", [1, DIM], f32, kind="ExternalInput")
    ident_d = nc.dram_tensor("ident", [128, 128], bf16, kind="ExternalInput")
    ind8_d = nc.dram_tensor("ind8", [8, DIM], bf16, kind="ExternalInput")
    # -15 * (cls self-term): rows 0:64 = exp(s_self_h)*v_cls_h, row 64 =
    # exp(s_self_h); column h per head. Host-computed.
    neg15_d = nc.dram_tensor("neg15", [65, 8], f32, kind="ExternalInput")
    out_d = nc.dram_tensor("out", [N_SEQ, DIM], f32, kind="ExternalOutput")

    NBUF = 3  # manual rotation depth for kq
    NVBUF = 4  # v_ext rotation depth

    with tile.TileContext(nc) as tc:
        with (
            tc.tile_pool(name="const", bufs=1) as cpool,
            tc.tile_pool(name="scr", bufs=2) as scr,
            tc.tile_pool(name="ps_s", bufs=2, space="PSUM") as ps_s_pool,
            tc.tile_pool(name="ps_poo", bufs=3, space="PSUM") as ps_poo_pool,
            tc.tile_pool(name="ps_misc", bufs=3, space="PSUM") as ps_misc_pool,
        ):
            # ---------------- persistent SBUF tensors ----------------
            xT = []
            for c in range(4):
                t = cpool.tile([128, N_SEQ], bf16, name=f"xT{c}", tag=f"xT{c}")
                nc.sync.dma_start(out=t[:], in_=xt_d[c * 128 : (c + 1) * 128, :])
                xT.append(t)
            wqk = []
            for c in range(4):
                t = cpool.tile([128, 2 * DIM], bf16, name=f"wqk{c}", tag=f"wqk{c}")
                nc.sync.dma_start(out=t[:], in_=wqk_d[c * 128 : (c + 1) * 128, :])
                wqk.append(t)
            wv = []
            for c in range(4):
                t = cpool.tile([128, DIM], bf16, name=f"wv{c}", tag=f"wv{c}")
                nc.sync.dma_start(out=t[:], in_=wv_d[c * 128 : (c + 1) * 128, :])
                wv.append(t)
            wout = []
            for c in range(4):
                t = cpool.tile([128, DIM], bf16, name=f"wout{c}", tag=f"wout{c}")
                nc.sync.dma_start(out=t[:], in_=wout_d[c * 128 : (c + 1) * 128, :])
                wout.append(t)
            bout_sb = cpool.tile([1, DIM], f32, name="bout", tag="bout")
            nc.sync.dma_start(out=bout_sb[:], in_=bout_d[:])
            ident = cpool.tile([128, 128], bf16, name="ident", tag="ident")
            nc.sync.dma_start(out=ident[:], in_=ident_d[:])
            ind8 = cpool.tile([8, DIM], bf16, name="ind8", tag="ind8")
            nc.sync.dma_start(out=ind8[:], in_=ind8_d[:])
            neg15 = cpool.tile([65, 8], f32, name="neg15", tag="neg15")
            nc.sync.dma_start(out=neg15[:], in_=neg15_d[:])

            ones_row_bf = cpool.tile([1, 128], bf16, name="ones_row_bf", tag="orbf")
            nc.gpsimd.memset(ones_row_bf[:], 1.0)
            bout_bf = cpool.tile([1, DIM], bf16, name="bout_bf", tag="bout_bf")
            nc.vector.tensor_copy(bout_bf[:], bout_sb[:])

            # global attnT buffers: [128 inner dims (2 heads), NTOK] bf16
            attnT = [
                cpool.tile([128, NTOK], bf16, name=f"attnT{c}", tag=f"attnT{c}")
                for c in range(4)
            ]

            # rotating buffer sets (manual, so one-time inits survive reuse)
            kq_sets = [
                [
                    cpool.tile([128, 2, NKP], bf16, name=f"kq{s}_{m}", tag=f"kq{s}_{m}")
                    for m in range(8)
                ]
                for s in range(NBUF)
            ]
            v_sets = [
                (
                    cpool.tile([128, 8 * 65], bf16, name=f"v0_{s}", tag=f"v0_{s}"),
                    cpool.tile([128, 8 * 65], bf16, name=f"v1_{s}", tag=f"v1_{s}"),
                )
                for s in range(NVBUF)
            ]
            aT_sets = [
                cpool.tile([128, 2 * NQ], bf16, name=f"aT{s}", tag=f"aT{s}")
                for s in range(12)
            ]

            # S: per-frame staging strip [65, 8*197] bf16: per head
            # (attn-out rows 0:64 | den row 64) x (196 frame q | cls q col)
            S_sets = [
                cpool.tile([65, 8 * NQ], bf16, name=f"S_{p}", tag=f"S_{p}")
                for p in range(2)
            ]
            # cls stash: per-frame [65, 8] slices, reduced at epilogue
            stash = cpool.tile([65, F * 8], bf16, name="stash", tag="stash")
            s8_sets = [
                cpool.tile([8, NF], bf16, name=f"s8_{s}", tag=f"s8_{s}")
                for s in range(2)
            ]
            rs8_sets = [
                cpool.tile([8, NF], bf16, name=f"rs8_{s}", tag=f"rs8_{s}")
                for s in range(2)
            ]
            lden_sets = [
                cpool.tile([8, NF], f32, name=f"lden_{s}", tag=f"lden_{s}")
                for s in range(2)
            ]

            # ---------------- one-time inits on rotating sets ----------------
            for s in range(NVBUF):
                v0, v1 = v_sets[s]
                nc.gpsimd.memset(v1[64:128, :], 0.0)
                nc.gpsimd.memset(
                    v0[:].rearrange("p (h c) -> p h c", c=65)[:, :, 64:65], 1.0
                )
                nc.gpsimd.memset(
                    v1[0:69, :].rearrange("p (h c) -> p h c", c=65)[:, :, 64:65], 1.0
                )
            for s in range(NBUF):
                for m in range(4, 8):
                    for fl in range(2):
                        nc.gpsimd.memset(kq_sets[s][m][:, fl, NK:NKP], 0.0)

            # ---------------- emit helpers ----------------
            def emit_kq_proj_group(fp, m):
                pr0 = 1 + fp * 2 * NF
                ps = ps_misc_pool.tile([128, 2 * NF], f32, name="ps_kq", tag="misc")
                for c in range(4):
                    nc.tensor.matmul(
                        ps[:],
                        lhsT=wqk[c][:, m * 128 : (m + 1) * 128],
                        rhs=xT[c][:, pr0 : pr0 + 2 * NF],
                        start=(c == 0),
                        stop=(c == 3),
                    )
                return ps

            def emit_kq_copy(ps, m, dst_set, use_act):
                kqt = kq_sets[dst_set][m]
                dst = kqt[:, :, 0:NF]
                src = ps[:].rearrange("p (a b) -> p a b", b=NF)
                if use_act:
                    nc.scalar.copy(dst, src)
                else:
                    nc.vector.tensor_copy(dst, src)

            def emit_v_proj(f):
                r0 = 1 + f * NF
                out_ps = []
                for t, (t0, tn) in enumerate(((0, 128), (128, 68))):
                    ps = ps_misc_pool.tile([tn, DIM], f32, name="ps_v", tag="misc")
                    for c in range(4):
                        nc.tensor.matmul(
                            ps[:],
                            lhsT=xT[c][:, r0 + t0 : r0 + t0 + tn],
                            rhs=wv[c][:],
                            start=(c == 0),
                            stop=(c == 3),
                        )
                    out_ps.append((ps, t, tn))
                return out_ps

            def emit_v_copy(ps, t, tn, vset):
                vx = v_sets[vset][t]
                nc.vector.tensor_copy(
                    vx[0:tn, :].rearrange("p (h c) -> p h c", c=65)[:, :, 0:64],
                    ps[:].rearrange("p (h c) -> p h c", c=64),
                )

            och_emitted = [False] * N_OCH

            def ready_ochunks(f_done):
                lim = (f_done + 1) * NF
                out = []
                for k in range(N_OCH):
                    if not och_emitted[k] and min((k + 1) * 128, NTOK) <= lim:
                        och_emitted[k] = True
                        out.append(k)
                return out

            def emit_outproj(k):
                t0 = k * 128
                tn = min(128, NTOK - t0)
                ps = ps_poo_pool.tile([tn, DIM], f32, name="ps_o", tag="poo")
                for c in range(4):
                    nc.tensor.matmul(
                        ps[:],
                        lhsT=attnT[c][:, t0 : t0 + tn],
                        rhs=wout[c][:],
                        start=(c == 0),
                        stop=(c == 3 and not with_bias),
                    )
                if with_bias:
                    nc.tensor.matmul(
                        ps[:], lhsT=ones_row_bf[0:1, 0:tn], rhs=bout_bf[:],
                        start=False, stop=True,
                    )
                o_sb = scr.tile([tn, DIM], f32, name="osb", tag=f"osb{k % 2}")
                nc.vector.tensor_copy(o_sb[:], ps[:])
                nc.sync.dma_start(out=out_d[1 + t0 : 1 + t0 + tn, :], in_=o_sb[:])

            def emit_recip(f, s8t, rs8t):
                # 1/x as exp(-ln(x)) on the Act engine: ~0.7us vs 1.37us on
                # DVE, and off the busy DVE queue
                ld = lden_sets[f % 2]
                nc.scalar.activation(ld[:], s8t[:], AF.Ln)
                nc.scalar.activation(rs8t[:], ld[:], AF.Exp, scale=-1.0)

            def emit_norm(f, s8t, rs8t):
                t0 = f * NF
                for c in range(4):
                    ps_r = ps_misc_pool.tile([128, NF], f32, name="ps_r", tag="misc")
                    nc.tensor.matmul(
                        ps_r[:],
                        lhsT=ind8[:, c * 128 : (c + 1) * 128],
                        rhs=rs8t[:],
                        start=True,
                        stop=True,
                    )
                    nc.vector.tensor_mul(
                        attnT[c][:, t0 : t0 + NF],
                        attnT[c][:, t0 : t0 + NF],
                        ps_r[:],
                    )

            # prologue: project pair 0 first (big 392-row groups ramp the PE)
            for m in range(8):
                ps = emit_kq_proj_group(0, m)
                emit_kq_copy(ps, m, 0, use_act=False)
            for f in (0, 1):
                for ps, t, tn in emit_v_proj(f):
                    emit_v_copy(ps, t, tn, f % NVBUF)

            # ---------------- software-pipelined main loop ----------------

            # ---------------- preamble: cls q/k/v projections ----------------
            # v_cls row [1, 512] fp32
            v_cls = cpool.tile([1, DIM], f32, name="v_cls", tag="v_cls")
            ps = ps_misc_pool.tile([1, DIM], f32, name="ps_vc", tag="misc")
            for c in range(4):
                nc.tensor.matmul(
                    ps[:], lhsT=xT[c][:, 0:1], rhs=wv[c][:],
                    start=(c == 0), stop=(c == 3),
                )
            nc.vector.tensor_copy(v_cls[:], ps[:])

            # qkT_cls[m]: [128, 1] bf16 (transposed cls q/k per 128-dim chunk)
            qkT_cls = []
            for m in range(8):
                ps = ps_misc_pool.tile([128, 1], f32, name="ps_qt", tag="misc")
                for c in range(4):
                    nc.tensor.matmul(
                        ps[:],
                        lhsT=wqk[c][:, m * 128 : (m + 1) * 128],
                        rhs=xT[c][:, 0:1],
                        start=(c == 0),
                        stop=(c == 3),
                    )
                t = cpool.tile([128, 1], bf16, name=f"qkTc{m}", tag=f"qkTc{m}")
                nc.vector.tensor_copy(t[:], ps[:])
                qkT_cls.append(t)

            # one-time: cls q and k columns into every kq buffer set (col 196)
            for s in range(NBUF):
                for m in range(8):
                    for fl in range(2):
                        nc.gpsimd.tensor_copy(
                            kq_sets[s][m][:, fl, NF : NF + 1], qkT_cls[m][:]
                        )

            # v_ext_cls [1, 520] bf16: cls v + per-head ones; one-time row 68
            v_ext_cls = cpool.tile([1, 8 * 65], bf16, name="v_ext_cls", tag="vec")
            nc.gpsimd.memset(
                v_ext_cls[:].rearrange("p (h c) -> p h c", c=65)[:, :, 64:65], 1.0
            )
            nc.vector.tensor_copy(
                v_ext_cls[:].rearrange("p (h c) -> p h c", c=65)[:, :, 0:64],
                v_cls[:].rearrange("p (h c) -> p h c", c=64),
            )
            for s in range(NVBUF):
                nc.sync.dma_start(out=v_sets[s][1][68:69, :], in_=v_ext_cls[:])


            pend_norm = None

            for f in range(F):
                fp, fl = f // 2, f % 2
                kset = fp % NBUF
                vset = f % NVBUF
                s8t = s8_sets[f % 2]
                rs8t = rs8_sets[f % 2]
                St = S_sets[f % 2]
                Sv = St[:].rearrange("p (h q) -> p h q", q=NQ)

                fillers = []
                if fp + 1 < F // 2:
                    nset = (fp + 1) % NBUF
                    ms = range(4) if fl == 0 else range(4, 8)
                    for m in ms:
                        fillers.append(("kq", m, nset))
                if f + 2 < F:
                    fillers.append(("v", f + 2, (f + 2) % NVBUF))

                def pop_filler():
                    if not fillers:
                        return
                    kind = fillers.pop(0)
                    if kind[0] == "kq":
                        _, m, nset = kind
                        ps = emit_kq_proj_group(fp + 1, m)
                        emit_kq_copy(ps, m, nset, use_act=False)
                    else:
                        _, vf, vs = kind
                        for ps, t, tn in emit_v_proj(vf):
                            emit_v_copy(ps, t, tn, vs)

                v0, v1 = v_sets[vset]
                aT_h = [None] * 8

                # start the previous frame's reciprocal ASAP so the norm
                # broadcast matmuls (emitted mid-frame) never stall the PE
                if pend_norm is not None:
                    emit_recip(*pend_norm)

                def emit_scores(h):
                    r = (h % 2) * 64
                    c = h // 2
                    ps = ps_s_pool.tile([128, 2 * NQ], f32, name="ps_sc", tag="sc")
                    nc.tensor.matmul(
                        ps[:, 0:NQ],
                        lhsT=kq_sets[kset][4 + c][r : r + 64, fl, 0:128],
                        rhs=kq_sets[kset][c][r : r + 64, fl, 0:NQ],
                        start=True,
                        stop=True,
                    )
                    # keys 128:198 = 68 frame + cls + 1 pad (trim stationary)
                    nc.tensor.matmul(
                        ps[0:70, NQ : 2 * NQ],
                        lhsT=kq_sets[kset][4 + c][r : r + 64, fl, 128:198],
                        rhs=kq_sets[kset][c][r : r + 64, fl, 0:NQ],
                        start=True,
                        stop=True,
                    )
                    aT = aT_sets[(f % 3) * 4 + (h % 4)]
                    nc.scalar.activation(aT[:], ps[:], AF.Exp)
                    aT_h[h] = aT

                def emit_av(h):
                    aT = aT_h[h]
                    po = ps_poo_pool.tile([65, NQ], f32, name="po", tag="poo")
                    nc.tensor.matmul(
                        po[:], lhsT=v0[:, h * 65 : (h + 1) * 65], rhs=aT[:, 0:NQ],
                        start=True, stop=False,
                    )
                    nc.tensor.matmul(
                        po[:], lhsT=v1[:, h * 65 : (h + 1) * 65],
                        rhs=aT[:, NQ : 2 * NQ],
                        start=False, stop=True,
                    )
                    # drain into the frame staging strip
                    if h % 2 == 0:
                        nc.scalar.copy(Sv[:, h, :], po[:])
                    else:
                        nc.vector.tensor_copy(Sv[:, h, :], po[:])
                    # stage unnormalized attnT on the idle gpsimd DSP
                    r = (h % 2) * 64
                    c = h // 2
                    nc.gpsimd.tensor_copy(
                        attnT[c][r : r + 64, f * NF : (f + 1) * NF],
                        Sv[0:64, h, 0:NF],
                    )

                pop_filler()
                emit_scores(0)
                emit_scores(1)
                emit_scores(2)
                pop_filler()
                emit_av(0)
                emit_scores(3)
                pop_filler()
                emit_av(1)
                emit_scores(4)
                if pend_norm is not None:
                    emit_norm(*pend_norm)
                    pend_norm = None
                emit_av(2)
                emit_scores(5)
                pop_filler()
                emit_av(3)
                emit_scores(6)
                pop_filler()
                emit_av(4)
                emit_scores(7)
                emit_av(5)
                emit_av(6)
                emit_av(7)
                # out-proj after the frame's po allocations so its psum-ring
                # slot never blocks an av matmul mid-frame
                if f >= 1:
                    for k in ready_ochunks(f - 1):
                        emit_outproj(k)
                while fillers:
                    pop_filler()

                # per-frame gathers: denominators [8, 196] and cls (num|den)
                # columns [65, 8] -> stash slice
                nc.sync.dma_start(out=s8t[:], in_=Sv[64:65, :, 0:NF])
                nc.sync.dma_start(
                    out=stash[:].rearrange("p (f h) -> p f h", h=8)[:, f, :],
                    in_=Sv[:, :, NF],
                )
                pend_norm = (f, s8t, rs8t)

            emit_recip(*pend_norm)
            emit_norm(*pend_norm)
            for k in ready_ochunks(F - 1):
                emit_outproj(k)

            # ---------------- cls epilogue ----------------
            acc65 = scr.tile([65, 8], f32, name="acc65", tag="acc65")
            nc.vector.tensor_copy(acc65[:], neg15[:])
            for f0 in range(F):
                nc.vector.tensor_add(
                    acc65[:], acc65[:],
                    stash[:].rearrange("p (a h) -> p a h", h=8)[:, f0, :],
                )
            accT = ps_misc_pool.tile([8, 65], bf16, name="accT", tag="misc")
            acc_bf = scr.tile([65, 8], bf16, name="acc_bf", tag="acc_bf")
            nc.vector.tensor_copy(acc_bf[:], acc65[:])
            nc.tensor.transpose(accT[:], acc_bf[:], ident[0:65, 0:65])
            accT_sb = scr.tile([8, 65], f32, name="accT_sb", tag="accT_sb")
            nc.vector.tensor_copy(accT_sb[:], accT[:])
            rden = scr.tile([8, 1], f32, name="rden", tag="rden")
            nc.vector.reciprocal(rden[:], accT_sb[:, 64:65])
            cls_n = scr.tile([8, 64], bf16, name="cls_n", tag="cls_n")
            nc.vector.tensor_scalar_mul(cls_n[:], accT_sb[:, 0:64], rden[:, 0:1])
            ps_t = ps_misc_pool.tile([64, 8], bf16, name="ps_t", tag="misc")
            nc.tensor.transpose(ps_t[:], cls_n[:], ident[0:8, 0:8])
            attnT_cls = [
                scr.tile([128, 1], bf16, name=f"aTc{c}", tag=f"aTc{c}")
                for c in range(4)
            ]
            for h in range(8):
                nc.vector.tensor_copy(
                    attnT_cls[h // 2][(h % 2) * 64 : (h % 2) * 64 + 64, :],
                    ps_t[:, h : h + 1],
                )
            ps_oc = ps_poo_pool.tile([1, DIM], f32, name="ps_oc", tag="poo")
            for c in range(4):
                nc.tensor.matmul(
                    ps_oc[:], lhsT=attnT_cls[c][:], rhs=wout[c][:],
                    start=(c == 0), stop=(c == 3),
                )
            o_cls = scr.tile([1, DIM], f32, name="o_cls", tag="o_cls")
            nc.vector.tensor_add(o_cls[:], ps_oc[:], bout_sb[:])
            nc.sync.dma_start(out=out_d[0:1, :], in_=o_cls[:])

    return nc


_NC_CACHE = {}


def _get_nc(with_bias):
    key = f"nc{int(with_bias)}"
    if key not in _NC_CACHE:
        _NC_CACHE[key] = build_kernel(with_bias)
    return _NC_CACHE[key]


def kernel(x, Wqkv, Wout, bout, f, _trace=False, _trace_kwargs=None):
    assert int(f) == F, f"kernel hardcoded for f={F}, got {f}"
    import ml_dtypes

    x = np.asarray(x, np.float32)
    Wqkv_s = np.asarray(Wqkv, np.float32).copy()
    Wqkv_s[:, :DIM] *= DH ** -0.5  # fold q scaling into the projection
    wqk_np = np.ascontiguousarray(Wqkv_s[:, : 2 * DIM]).astype(ml_dtypes.bfloat16)
    wv_np = np.ascontiguousarray(Wqkv_s[:, 2 * DIM :]).astype(ml_dtypes.bfloat16)
    wout_np = np.asarray(Wout, np.float32).astype(ml_dtypes.bfloat16)
    bout2 = np.asarray(bout, np.float32).reshape(1, DIM)

    ident_np = np.eye(128, dtype=ml_dtypes.bfloat16)
    ind8_np = np.zeros((8, DIM), dtype=ml_dtypes.bfloat16)
    for k in range(8):
        ind8_np[k, k * 64 : (k + 1) * 64] = 1.0

    xt_all = np.ascontiguousarray(x.transpose(0, 2, 1)).astype(ml_dtypes.bfloat16)

    # host-computed -15 * cls self-term per batch (fp32): the device counts
    # the cls self-attention term once per frame block (16x); subtract 15.
    x_cls = x[:, 0, :]  # [B, 512]
    q_cls = x_cls @ Wqkv_s[:, :DIM]  # scaled q  [B, 512]
    k_cls = x_cls @ Wqkv_s[:, DIM : 2 * DIM]
    v_cls = x_cls @ Wqkv_s[:, 2 * DIM :]
    qh = q_cls.reshape(-1, 8, 64)
    kh = k_cls.reshape(-1, 8, 64)
    vh = v_cls.reshape(-1, 8, 64)
    s_self = np.einsum("bhd,bhd->bh", qh, kh)  # [B, 8]
    e_self = np.exp(s_self)
    neg15_np = np.zeros((x.shape[0], 65, 8), np.float32)
    neg15_np[:, 0:64, :] = -15.0 * (e_self[:, None, :] * vh.transpose(0, 2, 1))
    neg15_np[:, 64, :] = -15.0 * e_self

    with_bias = bool(np.any(np.asarray(bout)))
    nc = _get_nc(with_bias)
    in_maps = [
        {
            "xt": xt_all[i],
            "wqk": wqk_np,
            "wv": wv_np,
            "wout": wout_np,
            "bout": bout2,
            "ident": ident_np,
            "ind8": ind8_np,
            "neg15": neg15_np[i],
        }
        for i in range(N_CORES)
    ]
    res = run_bass_kernel_spmd(
        nc,
        in_maps,
        list(range(N_CORES)),
        trace=_trace,
        **(_trace_kwargs or {}),
    )
    out = np.stack([res.results[i]["out"] for i in range(N_CORES)], axis=0)
    if _trace:
        kernel.last_results = res
    return out
